# revision 1
# baseline (speedup 1.0000x reference)
"""Causal self-attention (B=4, T=2048, C=2048, H=16, HD=128) on 8 trn2 cores.

Sharding: core c handles batch b = c//2 and heads (c%2)*8 .. +8.
  - QKV projection column-sharded by head, attention head-sharded,
    c_proj row-sharded; the pair partial sums are combined on host.

v3: all matmul operands bf16 (same PE rate as f32r in the cost model,
half the DMA/SBUF), fully SBUF-resident pipeline (no q/k/v DRAM spill),
fused per-head schedule: project head h+1 while attention for head h
runs on the PE; RoPE on DVE overlaps attention; flash tiles of 256
queries to trim the causal diagonal.

Self-contained: hardcodes shapes; builds one SPMD Bass program and runs
it on cores 0-7 via run_bass_kernel_spmd.
"""
import math

import ml_dtypes
import numpy as np

import concourse.bass as bass
import concourse.bass_isa as bass_isa
import concourse.library_config as library_config
import concourse.mybir as mybir
import concourse.tile as tile
from concourse.bass_utils import run_bass_kernel_spmd

F32 = mybir.dt.float32
BF16 = mybir.dt.bfloat16
AF = mybir.ActivationFunctionType
ALU = mybir.AluOpType
NPBF = ml_dtypes.bfloat16

# problem dims
B, T, C, H = 4, 2048, 2048, 16
HD = 128
NCORES = 8
NH = H // 2          # heads per core

_ctr = [0]


def _legalize_waits(nc, max_waits=1):
    """This walrus build rejects >1 sync wait per instruction. Hoist extra
    waits onto same-engine NoOps inserted directly before the instruction."""
    n_split = 0
    for f in nc.m.functions:
        for blk in f.blocks:
            newil = []
            changed = False
            for inst in blk.instructions:
                si = inst.sync_info
                if si is not None and si.on_wait and len(si.on_wait) > max_waits:
                    waits = list(si.on_wait)
                    for w in waits[:-max_waits]:
                        _ctr[0] += 1
                        nop = mybir.InstNoOp(name=f"I-waitfix-{_ctr[0]}")
                        nop.engine = inst.engine
                        nop.sync_info = mybir.SyncInfo(on_wait=[w], on_update=[])
                        newil.append(nop)
                    inst.sync_info = mybir.SyncInfo(
                        on_wait=waits[-max_waits:], on_update=list(si.on_update)
                    )
                    changed = True
                    n_split += 1
                newil.append(inst)
            if changed:
                blk.instructions = newil
    return n_split


def build_program(T=T, C=C, NH=NH, use_bqkv=False, qtile=256, legalize=True):
    """One core's program: full pipeline for (1 batch, NH heads)."""
    CB = C // 128          # contraction blocks
    TBn = T // 128         # token blocks
    QTILE = min(qtile, T)  # flash query-tile
    NQT = T // QTILE
    JMAX = QTILE // 128
    DV = NH * 128          # v/proj-shard width
    NG = max(1, DV // 512)  # v-projection head groups (512 cols each)
    GW = DV // NG           # group width
    GH = NH // NG           # heads per group
    TCH = 512               # xt column chunk / qk psq chunk
    NTC = T // TCH
    inv_sqrt_hd = 1.0 / math.sqrt(HD)

    nc = bass.Bass()
    xt_d = nc.dram_tensor("xt", [128, CB, T], BF16, kind="ExternalInput")
    wqk_d = nc.dram_tensor("wqk", [2, NH, 128, C], BF16, kind="ExternalInput")
    wv_d = nc.dram_tensor("wv", [128, CB, DV], BF16, kind="ExternalInput")
    wp_d = nc.dram_tensor("wp", [NH, 128, C], BF16, kind="ExternalInput")
    cos2_d = nc.dram_tensor("cos2", [128, T], BF16, kind="ExternalInput")
    sin2s_d = nc.dram_tensor("sin2s", [128, T], BF16, kind="ExternalInput")
    mask_d = nc.dram_tensor("maskbig", [128, 2 * QTILE - 128], BF16, kind="ExternalInput")
    ones_d = nc.dram_tensor("ones128", [128, 128], BF16, kind="ExternalInput")
    if use_bqkv:
        # [128, 2*NH] per-partition q/k bias columns; V bias via rank-1 matmul
        bqk_d = nc.dram_tensor("bqk", [128, 2 * NH], F32, kind="ExternalInput")
        onecol_d = nc.dram_tensor("onecol", [1, 128], BF16, kind="ExternalInput")
        bv_d = nc.dram_tensor("bv", [1, DV], BF16, kind="ExternalInput")
    out_d = nc.dram_tensor("out_partial", [T, C], BF16, kind="ExternalOutput")

    with tile.TileContext(nc) as tc:
        cms = {}

        def openpool(name, **kw):
            cm = tc.tile_pool(name=name, bufs=1, **kw)
            cms[name] = cm
            return cm.__enter__()

        def close(*names):
            for n in names:
                cms.pop(n).__exit__(None, None, None)

        # ---- pools + tiles up front, in per-side stack order.
        # left SBUF stack (live to the end): cpool..ropool;
        # right stack: xpool, later replaced by wppool.
        cpool = openpool("cpool")
        cos2 = cpool.tile([128, T], BF16, name="cos2")
        sin2s = cpool.tile([128, T], BF16, name="sin2s")
        maskt = cpool.tile([128, 2 * QTILE - 128], BF16, name="maskt")
        ones = cpool.tile([128, 128], BF16, name="ones")
        if use_bqkv:
            bqk = cpool.tile([128, 2 * NH], F32, name="bqk")
            onecol = cpool.tile([1, 128], BF16, name="onecol")
            bv = cpool.tile([1, DV], BF16, name="bv")

        outc_r = [cpool.tile([128, 512], BF16, name=f"outc{i}", tag=f"outc{i}")
                  for i in range(2)]

        ohpool = openpool("ohpool")
        ohs = [ohpool.tile([128, T], BF16, name=f"oh{h}", tag=f"oh{h}")
               for h in range(NH)]

        wqpool = openpool("wqpool")
        wq_r = [(wqpool.tile([128, C], BF16, name=f"wq{i}", tag=f"wq{i}"),
                 wqpool.tile([128, C], BF16, name=f"wk{i}", tag=f"wk{i}"))
                for i in range(2)]

        vpool = openpool("vpool")
        vgs = [[vpool.tile([128, GW], BF16, name=f"vg{g}_{tb}", tag=f"vg{g}_{tb}")
                for tb in range(TBn)] for g in range(NG)]

        qkpool = openpool("qkpool")
        qk_r = [(qkpool.tile([128, T], BF16, name=f"qr{i}", tag=f"qr{i}"),
                 qkpool.tile([128, T], BF16, name=f"kr{i}", tag=f"kr{i}"))
                for i in range(3)]

        ppool = openpool("ppool")
        qb_r = [ppool.tile([128, TCH], BF16, name=f"qb{i}", tag=f"qb{i}")
                for i in range(3)]
        qrot_r = [ppool.tile([128, TCH], BF16, name=f"qrot{i}", tag=f"qrot{i}")
                  for i in range(3)]
        pt_r = [ppool.tile([128, QTILE], BF16, name=f"pt{i}", tag=f"pt{i}")
                for i in range(6)]

        ropool = openpool("ropool")
        rec_r = [ropool.tile([128, QTILE], F32, name="rec0", tag="rec0")]
        racc_r = [ropool.tile([128, QTILE], BF16, name=f"racc{i}", tag=f"racc{i}")
                  for i in range(2)]


        wvpool = openpool("wvpool", side="right")
        wvg_t = wvpool.tile([128, CB, GW], BF16, name="wvg")

        xpool = openpool("xpool", side="right")
        xbig = xpool.tile([128, CB, T], BF16, name="xbig")
        xts = [xbig[:, cb, :] for cb in range(CB)]

        # PSUM: prologue uses psq(2)+psv(2); psv then closes and the
        # attention pools take its banks -> psq2+psS2+psO2+psR2 = 8 banks.
        psqp = openpool("psq", space="PSUM")
        psq_r = [psqp.tile([128, TCH], F32, name=f"psq{i}", tag=f"psq{i}")
                 for i in range(2)]
        psvp = openpool("psv", space="PSUM")
        psv_r = [psvp.tile([128, GW], F32, name=f"psv{i}", tag=f"psv{i}")
                 for i in range(2)]
        psS_r, psO_r = [], []

        ctr = {"psv": 0, "psq": 0, "psS": 0, "psO": 0, "qb": 0, "pt": 0,
               "rec": 0, "racc": 0, "psPsm": 0, "outc": 0}

        def ring(rs, key):
            t = rs[ctr[key] % len(rs)]
            ctr[key] += 1
            return t

        # ---------------- DMA preloads (issue order = queue order) ----------
        def load_wq(h):
            wq, wk = wq_r[h % 2]
            nc.sync.dma_start(out=wq[:], in_=wqk_d[0, h])
            nc.sync.dma_start(out=wk[:], in_=wqk_d[1, h])

        def load_wvg(g):
            h2 = CB // 2
            nc.sync.dma_start(out=wvg_t[:, 0:h2, :],
                              in_=wv_d[:, 0:h2, g * GW:(g + 1) * GW])
            nc.sync.dma_start(out=wvg_t[:, h2:CB, :],
                              in_=wv_d[:, h2:CB, g * GW:(g + 1) * GW])

        def load_xt_chunk(tc_, fine=False):
            if fine:
                # per-cb pieces: compute can start as each lands
                for cb in range(CB):
                    nc.sync.dma_start(out=xbig[:, cb, tc_ * TCH:(tc_ + 1) * TCH],
                                      in_=xt_d[:, cb, tc_ * TCH:(tc_ + 1) * TCH])
            else:
                nc.sync.dma_start(out=xbig[:, :, tc_ * TCH:(tc_ + 1) * TCH],
                                  in_=xt_d[:, :, tc_ * TCH:(tc_ + 1) * TCH])

        nc.sync.dma_start(out=wq_r[0][0][:], in_=wqk_d[0, 0])
        load_xt_chunk(0, fine=True)
        nc.sync.dma_start(out=wq_r[0][1][:], in_=wqk_d[1, 0])
        load_wvg(0)
        load_xt_chunk(1, fine=True)
        nc.sync.dma_start(out=cos2[:], in_=cos2_d[:])
        nc.sync.dma_start(out=sin2s[:], in_=sin2s_d[:])
        for tc_ in range(2, NTC):
            load_xt_chunk(tc_)
        nc.sync.dma_start(out=maskt[:], in_=mask_d[:])
        nc.sync.dma_start(out=ones[:], in_=ones_d[:])
        load_wq(1)
        if use_bqkv:
            nc.sync.dma_start(out=bqk[:], in_=bqk_d[:])
            nc.sync.dma_start(out=onecol[:], in_=onecol_d[:])
            nc.sync.dma_start(out=bv[:], in_=bv_d[:])

        # ---------------- building blocks ----------------
        def vproj_group(g, tb0, tb1):
            """V columns for head group g, token blocks [tb0, tb1)."""
            for tb in range(tb0, tb1):
                psv = ring(psv_r, "psv")
                for cb in range(CB):
                    nc.tensor.matmul(
                        psv[:], xts[cb][:, tb * 128:(tb + 1) * 128], wvg_t[:, cb, :],
                        start=(cb == 0), stop=(cb == CB - 1 and not use_bqkv))
                if use_bqkv:
                    nc.tensor.matmul(psv[:], onecol[:], bv[:, g * GW:(g + 1) * GW],
                                     start=False, stop=True)
                nc.scalar.copy(out=vgs[g][tb][:], in_=psv[:])

        def rope_tail(h, s, tc_, ps, dmae=None):
            """PSUM chunk -> RoPE -> qr/kr slice (ACT+DMA+DVE, no PE work)."""
            dst = qk_r[h % 3][s]
            ts = slice(tc_ * TCH, (tc_ + 1) * TCH)
            qb = ring(qb_r, "qb")
            qrot = qrot_r[(ctr["qb"] - 1) % len(qrot_r)]
            if use_bqkv:
                nc.vector.tensor_scalar(
                    qb[:], ps[:], bqk[:, s * NH + h:s * NH + h + 1], None, ALU.add)
            else:
                nc.scalar.copy(out=qb[:], in_=ps[:])
            # partition-half swap.  During the prologue the SP queue is
            # congested with preloads -> use the idle gpsimd queue; later the
            # gpsimd queue carries the denominator all-reduces -> use SP.
            dmae = dmae or nc.sync
            dmae.dma_start(out=qrot[0:64, :], in_=qb[64:128, :])
            dmae.dma_start(out=qrot[64:128, :], in_=qb[0:64, :])
            nc.vector.tensor_mul(qb[:], qb[:], cos2[:, ts])
            nc.vector.tensor_mul(qrot[:], qrot[:], sin2s[:, ts])
            nc.vector.tensor_add(dst[:, ts], qb[:], qrot[:])

        def qkproj_chunk(h, s, tc_):
            """psq for (head h, q/k s), token chunk tc_, then RoPE.
            (prologue-only path: swaps ride the idle gpsimd queue)"""
            w = wq_r[h % 2][s]
            ts = slice(tc_ * TCH, (tc_ + 1) * TCH)
            ps = ring(psq_r, "psq")
            for cb in range(CB):
                nc.tensor.matmul(ps[:], w[:, cb * 128:(cb + 1) * 128],
                                 xts[cb][:, ts], start=(cb == 0), stop=(cb == CB - 1))
            rope_tail(h, s, tc_, ps, dmae=nc.gpsimd)

        def qkproj_head(h):
            for tc_ in range(NTC):
                qkproj_chunk(h, 0, tc_)
                qkproj_chunk(h, 1, tc_)

        def proj_gen(h):
            """Generator form of qkproj_head: yields after each PE matmul so
            projection work can be woven into an attention stream."""
            for tc_ in range(NTC):
                for s in (0, 1):
                    w = wq_r[h % 2][s]
                    ts = slice(tc_ * TCH, (tc_ + 1) * TCH)
                    ps = ring(psq_r, "psq")
                    for cb in range(CB):
                        nc.tensor.matmul(
                            ps[:], w[:, cb * 128:(cb + 1) * 128], xts[cb][:, ts],
                            start=(cb == 0), stop=(cb == CB - 1))
                        yield
                    rope_tail(h, s, tc_, ps)

        pending = []   # deferred per-qt R matmuls + norms, shared across heads

        def attn_gen(h, defer=True):
            """Generator: one flash-attention head; yields after each key
            block.  S matmuls run LOOKAHEAD blocks ahead of their O
            consumers so the exp result is ready before the PE needs it --
            the PE must run back-to-back to hold its top p-state.

            Softmax denominators never touch the PE: exp tiles accumulate
            via bf16 DVE adds into racc; a per-query-tile gpsimd
            partition_all_reduce produces the denominator, broadcast to all
            partitions, on the otherwise idle Pool engine."""
            LOOKAHEAD = 2
            qr, kr = qk_r[h % 3]
            g, hi = divmod(h, GH)
            vg = vgs[g]

            for qt in range(NQT):
                ntk = (qt + 1) * JMAX
                tqs = slice(qt * QTILE, (qt + 1) * QTILE)
                psO = ring(psO_r, "psO")
                racc = ring(racc_r, "racc")
                pts = {}

                def emit_S(tkb, ntk=ntk, qt=qt, pts=pts, racc=racc):
                    psS = ring(psS_r, "psS")
                    nc.tensor.matmul(
                        psS[:], kr[:, tkb * 128:(tkb + 1) * 128], qr[:, tqs],
                        start=True, stop=True)
                    pt = ring(pt_r, "pt")
                    nc.scalar.activation(pt[:], psS[:], AF.Exp, scale=inv_sqrt_hd)
                    j = tkb - qt * JMAX
                    if j >= 0:
                        m0 = (JMAX - 1 - j) * 128
                        nc.vector.tensor_mul(pt[:], pt[:], maskt[:, m0:m0 + QTILE])
                    pts[tkb] = pt
                    if tkb >= 1:
                        a = pts[0] if tkb == 1 else racc
                        nc.vector.tensor_add(racc[:], a[:], pt[:])

                def emit_O(tkb, ntk=ntk, pts=pts, psO=psO):
                    pt = pts[tkb]
                    nc.tensor.matmul(psO[:], vg[tkb][:, hi * 128:(hi + 1) * 128],
                                     pt[:], start=(tkb == 0), stop=(tkb == ntk - 1))

                for tkb in range(ntk):
                    emit_S(tkb)
                    if tkb >= LOOKAHEAD:
                        emit_O(tkb - LOOKAHEAD)
                    if pending and (fn := pending.pop(0)) is not None:
                        fn()
                    yield qt
                for tkb in range(max(0, ntk - LOOKAHEAD), ntk):
                    emit_O(tkb)
                    if pending and (fn := pending.pop(0)) is not None:
                        fn()
                    yield qt

                def denom_tail(racc=racc, psO=psO, tqs=tqs):
                    # single ones-matmul on the accumulated exp tile gives the
                    # denominator broadcast across partitions (psPsm banks are
                    # free outside the phase-C stream)
                    rP = ring(psPsm_r, "psPsm")
                    nc.tensor.matmul(rP[:, 0:QTILE], ones[:], racc[:],
                                     start=True, stop=True)
                    rec = ring(rec_r, "rec")
                    nc.vector.reciprocal(rec[:], rP[:, 0:QTILE])
                    nc.vector.tensor_mul(ohs[h][:, tqs], psO[:], rec[:])

                if defer:
                    pending.extend([None] * min(4, max(2, ntk // 4)) + [denom_tail])
                else:
                    denom_tail()

        def run_attn(h, filler=None):
            """Emit attention head h, weaving in filler matmuls (avg 1.5 per
            key block, so the filler spans the whole head) to keep the PE fed
            across the exp latency chain."""
            for i, _ in enumerate(attn_gen(h, defer=True)):
                if filler is not None:
                    for _ in range(2):
                        if next(filler, "done") == "done":
                            filler = None
                            break
            if filler is not None:
                for _ in filler:
                    pass

        psPsm_r = []

        def c_gen(wps):
            """Phase C emitter: psP chunk groups of 8 matmuls, then copy+DMA
            of that [128,512] output chunk. Yields its tb before each PE op."""
            for tb in range(TBn):
                rs = slice(tb * 128, (tb + 1) * 128)
                for c0 in range(0, C, 512):
                    psp = ring(psPsm_r, "psPsm")
                    for hd in range(NH):
                        yield tb
                        nc.tensor.matmul(
                            psp[:], ohs[hd][:, rs], wps[hd][:, c0:c0 + 512],
                            start=(hd == 0), stop=(hd == NH - 1))
                    ob = ring(outc_r, "outc")
                    nc.scalar.copy(out=ob[:], in_=psp[:])
                    nc.sync.dma_start(out=out_d[rs, c0:c0 + 512], in_=ob[:])

        def run_attn_pair_with_c(h0, h1, cg):
            """Interleave the last two attention heads block-by-block (each
            absorbs the other's softmax latency) and weave in phase-C chunk
            matmuls for query tiles whose normalization is already done."""
            g0, g1 = attn_gen(h0, defer=False), attn_gen(h1, defer=False)
            q0 = q1 = -1
            c_tb = next(cg)          # tb of the NEXT pending C matmul
            while True:
                step = False
                if q0 is not None:
                    q0 = next(g0, None)
                    step = step or q0 is not None
                if q1 is not None:
                    q1 = next(g1, None)
                    step = step or q1 is not None
                if not step:
                    break
                # pair norms are inline: qt i fully normed once both
                # generators are past it
                qmin = min(q0 if q0 is not None else NQT,
                           q1 if q1 is not None else NQT)
                for _ in range(4):
                    if c_tb is None or c_tb // JMAX + 1 > qmin:
                        break
                    c_tb = next(cg, None)
            # flush deferred norms, then drain the rest of phase C
            for fn in pending:
                if fn is not None:
                    fn()
            pending.clear()
            for _ in cg:
                pass

        # ---------------- prologue: v (all groups) + heads 0,1 projections --
        tbpg = TBn // NTC
        for tc_ in range(NTC):
            qkproj_chunk(0, 0, tc_)
            qkproj_chunk(0, 1, tc_)
            vproj_group(0, tc_ * tbpg, (tc_ + 1) * tbpg)
        if NH > 2:
            load_wq(2)
        if NG > 1:
            load_wvg(1)
            for tc_ in range(NTC):
                qkproj_chunk(1, 0, tc_)
                qkproj_chunk(1, 1, tc_)
                vproj_group(1, tc_ * tbpg, (tc_ + 1) * tbpg)
        else:
            qkproj_head(1)

        # swap psv banks for the attention accumulators + small phase-C psP:
        # psq(2) + psS(2) + psO(2) + psPsm(2) = 8 banks, static to the end
        close("psv")
        psSp = openpool("psS", space="PSUM")
        psS_r.extend(psSp.tile([128, QTILE], F32, name=f"psS{i}", tag=f"psS{i}")
                     for i in range(2))
        psOp = openpool("psO", space="PSUM")
        psO_r.extend(psOp.tile([128, QTILE], F32, name=f"psO{i}", tag=f"psO{i}")
                     for i in range(2))
        psPp2 = openpool("psPsm", space="PSUM")
        psPsm_r.extend(psPp2.tile([128, 512], F32, name=f"psPsm{i}", tag=f"psPsm{i}")
                       for i in range(2))

        # ---- steady state: attn(h) with head h+2's projection woven in ----
        for h in range(max(0, NH - 2)):
            if h + 3 < NH:
                load_wq(h + 3)
            run_attn(h, proj_gen(h + 2))

        # x is done (last read: head NH-1's projection); its SBUF hosts the
        # c_proj weights, whose DMA overlaps the last two attention heads
        close("xpool")
        wppool = tc.tile_pool(name="wppool", bufs=1, side="right")
        cms["wppool"] = wppool
        wppool_p = wppool.__enter__()
        wps = []
        for hd in range(NH):
            wpt = wppool_p.tile([128, C], BF16, name=f"wp{hd}", tag=f"wp{hd}")
            nc.sync.dma_start(out=wpt[:], in_=wp_d[hd])
            wps.append(wpt)

        # last heads: serial (block-interleaving two attention streams
        # miscompares on the hw path); phase C weaves into head NH-1 only,
        # gated on its query-tile progress (all other heads are done)
        run_attn(NH - 2)
        cg = c_gen(wps)
        c_tb = next(cg)
        for q in attn_gen(NH - 1, defer=False):
            for _ in range(4):
                if c_tb is None or c_tb // JMAX + 1 > q:
                    break
                c_tb = next(cg, None)
        for fn in pending:
            if fn is not None:
                fn()
        pending.clear()
        for _ in cg:
            pass

        close("psPsm", "psO", "psS", "psq",
              "wppool", "wvpool",
              "ropool", "ppool", "qkpool", "vpool",
              "wqpool", "ohpool", "cpool")

    if legalize:
        _legalize_waits(nc)
    return nc


# ---------------------------------------------------------------- host side

_PERM = np.concatenate([np.arange(0, HD, 2), np.arange(1, HD, 2)])  # de-interleave


def shard_core(core, x, freqs_cos, freqs_sin, Wqkv, bqkv, Wproj,
               T=T, C=C, NH=NH, qtile=256, use_bqkv=False):
    """Build the in_map for one core."""
    CB = C // 128
    DV = NH * 128
    QTILE = min(qtile, T)
    b = core // 2
    hb = (core % 2) * NH

    xt = np.ascontiguousarray(
        x[b].T.reshape(CB, 128, T).transpose(1, 0, 2)).astype(NPBF)

    # [2, NH, 128] column indices (q/k, de-interleaved within each head)
    cols = (np.arange(2)[:, None, None] * C
            + (hb + np.arange(NH))[None, :, None] * HD + _PERM[None, None, :])
    wqk = Wqkv[:, cols]                              # [C, 2, NH, 128]
    wqk = np.ascontiguousarray(
        wqk.reshape(CB, 128, 2, NH, 128).transpose(2, 3, 1, 0, 4)
        .reshape(2, NH, 128, C)).astype(NPBF)

    wv = np.ascontiguousarray(
        Wqkv[:, 2 * C + hb * HD: 2 * C + (hb + NH) * HD]
        .reshape(CB, 128, DV).transpose(1, 0, 2)).astype(NPBF)
    wp = np.ascontiguousarray(
        Wproj[hb * HD:(hb + NH) * HD, :].reshape(NH, 128, C)).astype(NPBF)

    cos2 = np.concatenate([freqs_cos.T, freqs_cos.T], 0)
    cos2 = np.ascontiguousarray(cos2).astype(NPBF)   # [128, T]
    sin2s = np.concatenate([-freqs_sin.T, freqs_sin.T], 0)
    sin2s = np.ascontiguousarray(sin2s).astype(NPBF)

    u = np.arange(2 * QTILE - 128)[None, :]
    p = np.arange(128)[:, None]
    maskbig = (p <= u - (QTILE - 128)).astype(NPBF)

    im = {
        "xt": xt, "wqk": wqk, "wv": wv, "wp": wp,
        "cos2": cos2, "sin2s": sin2s, "maskbig": maskbig,
        "ones128": np.ones((128, 128), NPBF),
    }
    if use_bqkv:
        bqk = np.empty((128, 2 * NH), np.float32)
        for s in range(2):
            for h in range(NH):
                bqk[:, s * NH + h] = bqkv[s * C + (hb + h) * HD + _PERM]
        im["bqk"] = bqk
        im["onecol"] = np.ones((1, 128), NPBF)
        im["bv"] = np.ascontiguousarray(
            bqkv[2 * C + hb * HD: 2 * C + (hb + NH) * HD][None, :]).astype(NPBF)
    return im


_CACHE = {}


def _get_program(use_bqkv):
    key = use_bqkv
    if key not in _CACHE:
        _CACHE[key] = build_program(use_bqkv=use_bqkv)
    return _CACHE[key]


def kernel(x, freqs_cos, freqs_sin, Wqkv, bqkv, Wproj, bproj):
    x = np.asarray(x, np.float32)
    freqs_cos = np.asarray(freqs_cos, np.float32)
    freqs_sin = np.asarray(freqs_sin, np.float32)
    Wqkv = np.asarray(Wqkv, np.float32)
    bqkv = np.asarray(bqkv, np.float32)
    Wproj = np.asarray(Wproj, np.float32)
    bproj = np.asarray(bproj, np.float32)

    use_bqkv = bool(np.any(bqkv != 0))
    nc = _get_program(use_bqkv)
    in_maps = [
        shard_core(c, x, freqs_cos, freqs_sin, Wqkv, bqkv, Wproj,
                   use_bqkv=use_bqkv)
        for c in range(NCORES)
    ]
    try:
        res = run_bass_kernel_spmd(nc, in_maps, list(range(NCORES))).results
    except Exception:
        # transient device faults have been observed; retry once
        res = run_bass_kernel_spmd(nc, in_maps, list(range(NCORES))).results

    out = np.empty((B, T, C), np.float32)
    for b in range(B):
        out[b] = (res[2 * b]["out_partial"].astype(np.float32)
                  + res[2 * b + 1]["out_partial"].astype(np.float32))
    out += bproj[None, None, :]
    return out



# revision 14
# speedup vs baseline: 1.1395x; 1.1395x over previous
"""Causal self-attention (B=4, T=2048, C=2048, H=16, HD=128) on 8 trn2 cores.

Sharding: core c handles batch b = c//2 and heads (c%2)*8 .. +8.
  - QKV projection column-sharded by head, attention head-sharded,
    c_proj row-sharded; the pair partial sums are combined on host.

v3: all matmul operands bf16 (same PE rate as f32r in the cost model,
half the DMA/SBUF), fully SBUF-resident pipeline (no q/k/v DRAM spill),
fused per-head schedule: project head h+1 while attention for head h
runs on the PE; RoPE on DVE overlaps attention; flash tiles of 256
queries to trim the causal diagonal.

v4: QKV/V projections in fp8e4 DoubleRow (0.5 cycles/row, K=256 per
instruction): x and the weights are split hi/lo into fp8 pairs ON THE
HOST (x ~= x_hi + x_lo with x_lo the fp8-rounding residual), and each
K-pair block issues three DoubleRow matmuls (hi*hi + lo*hi + hi*lo)
into the same PSUM group -- 0.75x the bf16 instruction cost at bf16-or-
better accuracy.  Weights are pre-scaled x64 so w_hi avoids the fp8
subnormal floor; the scale cancels via the exp() scale (q*k -> /4096)
and a host-side /64 on Wproj (v path).  Attention (S, O) and c_proj
stay bf16: their operands are produced on device, and splitting them
would cost more DVE/ACT time than the PE time saved.

Self-contained: hardcodes shapes; builds one SPMD Bass program and runs
it on cores 0-7 via run_bass_kernel_spmd.
"""
import math

import ml_dtypes
import numpy as np

import concourse.bass as bass
import concourse.bass_isa as bass_isa
import concourse.library_config as library_config
import concourse.mybir as mybir
import concourse.tile as tile
from concourse.bass_utils import run_bass_kernel_spmd

F32 = mybir.dt.float32
BF16 = mybir.dt.bfloat16
FP8 = mybir.dt.float8e4
AF = mybir.ActivationFunctionType
ALU = mybir.AluOpType
DR = mybir.MatmulPerfMode.DoubleRow
NPBF = ml_dtypes.bfloat16
NPF8 = mybir.dt.np(FP8)
WSCALE = 64.0            # fp8 weight pre-scale (keeps w_hi out of subnormals)

# problem dims
B, T, C, H = 4, 2048, 2048, 16
HD = 128
NCORES = 8
NH = H // 2          # heads per core

_ctr = [0]


def _legalize_waits(nc, max_waits=1):
    """This walrus build rejects >1 sync wait per instruction. Hoist extra
    waits onto same-engine NoOps inserted directly before the instruction."""
    n_split = 0
    for f in nc.m.functions:
        for blk in f.blocks:
            newil = []
            changed = False
            for inst in blk.instructions:
                si = inst.sync_info
                if si is not None and si.on_wait and len(si.on_wait) > max_waits:
                    waits = list(si.on_wait)
                    for w in waits[:-max_waits]:
                        _ctr[0] += 1
                        nop = mybir.InstNoOp(name=f"I-waitfix-{_ctr[0]}")
                        nop.engine = inst.engine
                        nop.sync_info = mybir.SyncInfo(on_wait=[w], on_update=[])
                        newil.append(nop)
                    inst.sync_info = mybir.SyncInfo(
                        on_wait=waits[-max_waits:], on_update=list(si.on_update)
                    )
                    changed = True
                    n_split += 1
                newil.append(inst)
            if changed:
                blk.instructions = newil
    return n_split


def build_program(T=T, C=C, NH=NH, use_bqkv=False, qtile=256, legalize=True):
    """One core's program: full pipeline for (1 batch, NH heads)."""
    CB = C // 128          # contraction blocks
    TBn = T // 128         # token blocks
    QTILE = min(qtile, T)  # flash query-tile
    NQT = T // QTILE
    JMAX = QTILE // 128
    DV = NH * 128          # v/proj-shard width
    NG = max(1, DV // 512)  # v-projection head groups (512 cols each)
    GW = DV // NG           # group width
    GH = NH // NG           # heads per group
    TCH = 512               # xt column chunk / qk psq chunk
    NTC = T // TCH
    inv_sqrt_hd = 1.0 / math.sqrt(HD)

    inv_sqrt_hd /= WSCALE * WSCALE   # q and k each carry a x64 weight scale

    nc = bass.Bass()
    xt_d = [nc.dram_tensor(f"xt{p}", [128, CB, T], FP8, kind="ExternalInput")
            for p in range(2)]
    wqk_d = [nc.dram_tensor(f"wqk{p}", [2, NH, 128, CB, 128], FP8,
                            kind="ExternalInput")
             for p in range(2)]
    wv_d = [nc.dram_tensor(f"wv{p}", [128, CB, DV], FP8, kind="ExternalInput")
            for p in range(2)]
    wp_d = nc.dram_tensor("wp", [NH, 128, C], BF16, kind="ExternalInput")
    cos2_d = nc.dram_tensor("cos2", [128, T], BF16, kind="ExternalInput")
    sin2s_d = nc.dram_tensor("sin2s", [128, T], BF16, kind="ExternalInput")
    mask_d = nc.dram_tensor("maskbig", [128, 2 * QTILE - 128], BF16, kind="ExternalInput")
    ones_d = nc.dram_tensor("ones128", [128, 128], BF16, kind="ExternalInput")
    if use_bqkv:
        # [128, 2*NH] per-partition q/k bias columns; V bias via rank-1 matmul
        bqk_d = nc.dram_tensor("bqk", [128, 2 * NH], F32, kind="ExternalInput")
        onecol_d = nc.dram_tensor("onecol", [1, 128], BF16, kind="ExternalInput")
        bv_d = nc.dram_tensor("bv", [1, DV], BF16, kind="ExternalInput")
    out_d = nc.dram_tensor("out_partial", [T, C], BF16, kind="ExternalOutput")

    with tile.TileContext(nc) as tc:
        cms = {}

        def openpool(name, **kw):
            cm = tc.tile_pool(name=name, bufs=1, **kw)
            cms[name] = cm
            return cm.__enter__()

        def close(*names):
            for n in names:
                cms.pop(n).__exit__(None, None, None)

        # ---- pools + tiles up front, in per-side stack order.
        # left SBUF stack (live to the end): cpool..ropool;
        # right stack: xpool, later replaced by wppool.
        cpool = openpool("cpool")
        cos2 = cpool.tile([128, T], BF16, name="cos2")
        sin2s = cpool.tile([128, T], BF16, name="sin2s")
        maskt = cpool.tile([128, 2 * QTILE - 128], BF16, name="maskt")
        ones = cpool.tile([128, 128], BF16, name="ones")
        if use_bqkv:
            bqk = cpool.tile([128, 2 * NH], F32, name="bqk")
            onecol = cpool.tile([1, 128], BF16, name="onecol")
            bv = cpool.tile([1, DV], BF16, name="bv")

        outc_r = [cpool.tile([128, 512], BF16, name=f"outc{i}", tag=f"outc{i}")
                  for i in range(2)]

        ohpool = openpool("ohpool")
        ohs = [ohpool.tile([128, T], BF16, name=f"oh{h}", tag=f"oh{h}")
               for h in range(NH)]

        wqpool = openpool("wqpool")
        # wq_r[slot][side(q/k)][part(hi/lo)] -> [128, CB, 128] fp8 tile
        wq_r = [[[wqpool.tile([128, CB, 128], FP8, name=f"w{s}{i}p{p}",
                              tag=f"w{s}{i}p{p}") for p in range(2)]
                 for s in range(2)] for i in range(2)]

        vpool = openpool("vpool")
        vgs = [[vpool.tile([128, GW], BF16, name=f"vg{g}_{tb}", tag=f"vg{g}_{tb}")
                for tb in range(TBn)] for g in range(NG)]

        qkpool = openpool("qkpool")
        qk_r = [(qkpool.tile([128, T], BF16, name=f"qr{i}", tag=f"qr{i}"),
                 qkpool.tile([128, T], BF16, name=f"kr{i}", tag=f"kr{i}"))
                for i in range(3)]

        ppool = openpool("ppool")
        qb_r = [ppool.tile([128, TCH], BF16, name=f"qb{i}", tag=f"qb{i}")
                for i in range(3)]
        qrot_r = [ppool.tile([128, TCH], BF16, name=f"qrot{i}", tag=f"qrot{i}")
                  for i in range(3)]
        pt_r = [ppool.tile([128, QTILE], BF16, name=f"pt{i}", tag=f"pt{i}")
                for i in range(6)]

        ropool = openpool("ropool")
        rec_r = [ropool.tile([128, QTILE], F32, name="rec0", tag="rec0")]
        racc_r = [ropool.tile([128, QTILE], BF16, name=f"racc{i}", tag=f"racc{i}")
                  for i in range(2)]


        wvpool = openpool("wvpool", side="right")
        wvg_t = [wvpool.tile([128, CB, GW], FP8, name=f"wvg{p}") for p in range(2)]

        xpool = openpool("xpool", side="right")
        xbig = [xpool.tile([128, CB, T], FP8, name=f"xbig{p}") for p in range(2)]

        # PSUM: prologue uses psq(2)+psv(2); psv then closes and the
        # attention pools take its banks -> psq2+psS2+psO2+psR2 = 8 banks.
        psqp = openpool("psq", space="PSUM")
        psq_r = [psqp.tile([128, TCH], F32, name=f"psq{i}", tag=f"psq{i}")
                 for i in range(2)]
        psvp = openpool("psv", space="PSUM")
        psv_r = [psvp.tile([128, GW], F32, name=f"psv{i}", tag=f"psv{i}")
                 for i in range(2)]
        psS_r, psO_r = [], []

        ctr = {"psv": 0, "psq": 0, "psS": 0, "psO": 0, "qb": 0, "pt": 0,
               "rec": 0, "racc": 0, "psPsm": 0, "outc": 0}

        def ring(rs, key):
            t = rs[ctr[key] % len(rs)]
            ctr[key] += 1
            return t

        # ---------------- DMA preloads (issue order = queue order) ----------
        def load_wq(h):
            for s in range(2):
                for p in range(2):
                    nc.sync.dma_start(out=wq_r[h % 2][s][p][:],
                                      in_=wqk_d[p][s, h])

        def load_wvg(g):
            h2 = CB // 2
            for p in range(2):
                nc.sync.dma_start(out=wvg_t[p][:, 0:h2, :],
                                  in_=wv_d[p][:, 0:h2, g * GW:(g + 1) * GW])
                nc.sync.dma_start(out=wvg_t[p][:, h2:CB, :],
                                  in_=wv_d[p][:, h2:CB, g * GW:(g + 1) * GW])

        def load_xt_chunk(tc_, fine=False):
            if fine:
                # per-cb-pair pieces: compute can start as each lands
                for cb in range(0, CB, 2):
                    for p in range(2):
                        nc.sync.dma_start(
                            out=xbig[p][:, cb:cb + 2, tc_ * TCH:(tc_ + 1) * TCH],
                            in_=xt_d[p][:, cb:cb + 2, tc_ * TCH:(tc_ + 1) * TCH])
            else:
                for p in range(2):
                    nc.sync.dma_start(
                        out=xbig[p][:, :, tc_ * TCH:(tc_ + 1) * TCH],
                        in_=xt_d[p][:, :, tc_ * TCH:(tc_ + 1) * TCH])

        for p in range(2):
            nc.sync.dma_start(out=wq_r[0][0][p][:], in_=wqk_d[p][0, 0])
        load_xt_chunk(0, fine=True)
        for p in range(2):
            nc.sync.dma_start(out=wq_r[0][1][p][:], in_=wqk_d[p][1, 0])
        load_wvg(0)
        load_xt_chunk(1, fine=True)
        nc.sync.dma_start(out=cos2[:], in_=cos2_d[:])
        nc.sync.dma_start(out=sin2s[:], in_=sin2s_d[:])
        for tc_ in range(2, NTC):
            load_xt_chunk(tc_)
        nc.sync.dma_start(out=maskt[:], in_=mask_d[:])
        nc.sync.dma_start(out=ones[:], in_=ones_d[:])
        load_wq(1)
        if use_bqkv:
            nc.sync.dma_start(out=bqk[:], in_=bqk_d[:])
            nc.sync.dma_start(out=onecol[:], in_=onecol_d[:])
            nc.sync.dma_start(out=bv[:], in_=bv_d[:])

        # ---------------- building blocks ----------------
        # fp8 DoubleRow term order per K-pair: hi*hi, lo*hi, hi*lo
        TERMS = ((0, 0), (1, 0), (0, 1))

        def vproj_group(g, tb0, tb1):
            """V columns for head group g, token blocks [tb0, tb1)."""
            for tb in range(tb0, tb1):
                psv = ring(psv_r, "psv")
                for cp in range(CB // 2):
                    for ti, (px, pw) in enumerate(TERMS):
                        nc.tensor.matmul(
                            psv[:],
                            xbig[px][:, 2 * cp:2 * cp + 2, tb * 128:(tb + 1) * 128],
                            wvg_t[pw][:, 2 * cp:2 * cp + 2, :],
                            start=(cp == 0 and ti == 0),
                            stop=(cp == CB // 2 - 1 and ti == 2 and not use_bqkv),
                            perf_mode=DR)
                if use_bqkv:
                    nc.tensor.matmul(psv[:], onecol[:], bv[:, g * GW:(g + 1) * GW],
                                     start=False, stop=True)
                nc.scalar.copy(out=vgs[g][tb][:], in_=psv[:])

        def rope_tail(h, s, tc_, ps, dmae=None):
            """PSUM chunk -> RoPE -> qr/kr slice (ACT+DMA+DVE, no PE work)."""
            dst = qk_r[h % 3][s]
            ts = slice(tc_ * TCH, (tc_ + 1) * TCH)
            qb = ring(qb_r, "qb")
            qrot = qrot_r[(ctr["qb"] - 1) % len(qrot_r)]
            if use_bqkv:
                nc.vector.tensor_scalar(
                    qb[:], ps[:], bqk[:, s * NH + h:s * NH + h + 1], None, ALU.add)
            else:
                nc.scalar.copy(out=qb[:], in_=ps[:])
            # partition-half swap.  During the prologue the SP queue is
            # congested with preloads -> use the idle gpsimd queue; later the
            # gpsimd queue carries the denominator all-reduces -> use SP.
            dmae = dmae or nc.sync
            dmae.dma_start(out=qrot[0:64, :], in_=qb[64:128, :])
            dmae.dma_start(out=qrot[64:128, :], in_=qb[0:64, :])
            nc.vector.tensor_mul(qb[:], qb[:], cos2[:, ts])
            nc.vector.tensor_mul(qrot[:], qrot[:], sin2s[:, ts])
            nc.vector.tensor_add(dst[:, ts], qb[:], qrot[:])

        def qk_matmuls(h, s, ts, ps):
            for cp in range(CB // 2):
                for ti, (px, pw) in enumerate(TERMS):
                    nc.tensor.matmul(
                        ps[:], wq_r[h % 2][s][pw][:, 2 * cp:2 * cp + 2, :],
                        xbig[px][:, 2 * cp:2 * cp + 2, ts],
                        start=(cp == 0 and ti == 0),
                        stop=(cp == CB // 2 - 1 and ti == 2),
                        perf_mode=DR)
                    yield

        def qkproj_chunk(h, s, tc_):
            """psq for (head h, q/k s), token chunk tc_, then RoPE.
            (prologue-only path: swaps ride the idle gpsimd queue)"""
            ts = slice(tc_ * TCH, (tc_ + 1) * TCH)
            ps = ring(psq_r, "psq")
            for _ in qk_matmuls(h, s, ts, ps):
                pass
            rope_tail(h, s, tc_, ps, dmae=nc.gpsimd)

        def qkproj_head(h):
            for tc_ in range(NTC):
                qkproj_chunk(h, 0, tc_)
                qkproj_chunk(h, 1, tc_)

        def proj_gen(h):
            """Generator form of qkproj_head: yields after each PE matmul so
            projection work can be woven into an attention stream."""
            for tc_ in range(NTC):
                for s in (0, 1):
                    ts = slice(tc_ * TCH, (tc_ + 1) * TCH)
                    ps = ring(psq_r, "psq")
                    yield from qk_matmuls(h, s, ts, ps)
                    rope_tail(h, s, tc_, ps)

        pending = []   # deferred per-qt R matmuls + norms, shared across heads

        def attn_gen(h, defer=True):
            """Generator: one flash-attention head; yields after each key
            block.  S matmuls run LOOKAHEAD blocks ahead of their O
            consumers so the exp result is ready before the PE needs it --
            the PE must run back-to-back to hold its top p-state.

            Softmax denominators never touch the PE: exp tiles accumulate
            via bf16 DVE adds into racc; a per-query-tile gpsimd
            partition_all_reduce produces the denominator, broadcast to all
            partitions, on the otherwise idle Pool engine."""
            LOOKAHEAD = 2
            qr, kr = qk_r[h % 3]
            g, hi = divmod(h, GH)
            vg = vgs[g]

            for qt in range(NQT):
                ntk = (qt + 1) * JMAX
                tqs = slice(qt * QTILE, (qt + 1) * QTILE)
                psO = ring(psO_r, "psO")
                racc = ring(racc_r, "racc")
                pts = {}

                def emit_S(tkb, ntk=ntk, qt=qt, pts=pts, racc=racc):
                    psS = ring(psS_r, "psS")
                    nc.tensor.matmul(
                        psS[:], kr[:, tkb * 128:(tkb + 1) * 128], qr[:, tqs],
                        start=True, stop=True)
                    pt = ring(pt_r, "pt")
                    nc.scalar.activation(pt[:], psS[:], AF.Exp, scale=inv_sqrt_hd)
                    j = tkb - qt * JMAX
                    if j >= 0:
                        m0 = (JMAX - 1 - j) * 128
                        nc.vector.tensor_mul(pt[:], pt[:], maskt[:, m0:m0 + QTILE])
                    pts[tkb] = pt
                    if tkb >= 1:
                        a = pts[0] if tkb == 1 else racc
                        nc.vector.tensor_add(racc[:], a[:], pt[:])

                def emit_O(tkb, ntk=ntk, pts=pts, psO=psO):
                    pt = pts[tkb]
                    nc.tensor.matmul(psO[:], vg[tkb][:, hi * 128:(hi + 1) * 128],
                                     pt[:], start=(tkb == 0), stop=(tkb == ntk - 1))

                for tkb in range(ntk):
                    emit_S(tkb)
                    if tkb >= LOOKAHEAD:
                        emit_O(tkb - LOOKAHEAD)
                    if pending and (fn := pending.pop(0)) is not None:
                        fn()
                    yield qt
                for tkb in range(max(0, ntk - LOOKAHEAD), ntk):
                    emit_O(tkb)
                    if pending and (fn := pending.pop(0)) is not None:
                        fn()
                    yield qt

                def denom_tail(racc=racc, psO=psO, tqs=tqs):
                    # single ones-matmul on the accumulated exp tile gives the
                    # denominator broadcast across partitions (psPsm banks are
                    # free outside the phase-C stream)
                    rP = ring(psPsm_r, "psPsm")
                    nc.tensor.matmul(rP[:, 0:QTILE], ones[:], racc[:],
                                     start=True, stop=True)
                    rec = ring(rec_r, "rec")
                    nc.vector.reciprocal(rec[:], rP[:, 0:QTILE])
                    nc.vector.tensor_mul(ohs[h][:, tqs], psO[:], rec[:])

                if defer:
                    pending.extend([None] * min(4, max(2, ntk // 4)) + [denom_tail])
                else:
                    denom_tail()

        def run_attn(h, filler=None):
            """Emit attention head h, weaving in filler matmuls (avg 1.5 per
            key block, so the filler spans the whole head) to keep the PE fed
            across the exp latency chain."""
            for i, _ in enumerate(attn_gen(h, defer=True)):
                if filler is not None:
                    for _ in range(2):
                        if next(filler, "done") == "done":
                            filler = None
                            break
            if filler is not None:
                for _ in filler:
                    pass

        psPsm_r = []

        def c_gen(wps):
            """Phase C emitter: psP chunk groups of 8 matmuls, then copy+DMA
            of that [128,512] output chunk. Yields its tb before each PE op."""
            for tb in range(TBn):
                rs = slice(tb * 128, (tb + 1) * 128)
                for c0 in range(0, C, 512):
                    psp = ring(psPsm_r, "psPsm")
                    for hd in range(NH):
                        yield tb
                        nc.tensor.matmul(
                            psp[:], ohs[hd][:, rs], wps[hd][:, c0:c0 + 512],
                            start=(hd == 0), stop=(hd == NH - 1))
                    ob = ring(outc_r, "outc")
                    nc.scalar.copy(out=ob[:], in_=psp[:])
                    nc.sync.dma_start(out=out_d[rs, c0:c0 + 512], in_=ob[:])

        def run_attn_pair_with_c(h0, h1, cg):
            """Interleave the last two attention heads block-by-block (each
            absorbs the other's softmax latency) and weave in phase-C chunk
            matmuls for query tiles whose normalization is already done."""
            g0, g1 = attn_gen(h0, defer=False), attn_gen(h1, defer=False)
            q0 = q1 = -1
            c_tb = next(cg)          # tb of the NEXT pending C matmul
            while True:
                step = False
                if q0 is not None:
                    q0 = next(g0, None)
                    step = step or q0 is not None
                if q1 is not None:
                    q1 = next(g1, None)
                    step = step or q1 is not None
                if not step:
                    break
                # pair norms are inline: qt i fully normed once both
                # generators are past it
                qmin = min(q0 if q0 is not None else NQT,
                           q1 if q1 is not None else NQT)
                for _ in range(4):
                    if c_tb is None or c_tb // JMAX + 1 > qmin:
                        break
                    c_tb = next(cg, None)
            # flush deferred norms, then drain the rest of phase C
            for fn in pending:
                if fn is not None:
                    fn()
            pending.clear()
            for _ in cg:
                pass

        # ---------------- prologue: v (all groups) + heads 0,1 projections --
        tbpg = TBn // NTC
        for tc_ in range(NTC):
            qkproj_chunk(0, 0, tc_)
            qkproj_chunk(0, 1, tc_)
            vproj_group(0, tc_ * tbpg, (tc_ + 1) * tbpg)
        if NH > 2:
            load_wq(2)
        if NG > 1:
            load_wvg(1)
            for tc_ in range(NTC):
                qkproj_chunk(1, 0, tc_)
                qkproj_chunk(1, 1, tc_)
                vproj_group(1, tc_ * tbpg, (tc_ + 1) * tbpg)
        else:
            qkproj_head(1)

        # swap psv banks for the attention accumulators + small phase-C psP:
        # psq(2) + psS(2) + psO(2) + psPsm(2) = 8 banks, static to the end
        close("psv")
        psSp = openpool("psS", space="PSUM")
        psS_r.extend(psSp.tile([128, QTILE], F32, name=f"psS{i}", tag=f"psS{i}")
                     for i in range(2))
        psOp = openpool("psO", space="PSUM")
        psO_r.extend(psOp.tile([128, QTILE], F32, name=f"psO{i}", tag=f"psO{i}")
                     for i in range(2))
        psPp2 = openpool("psPsm", space="PSUM")
        psPsm_r.extend(psPp2.tile([128, 512], F32, name=f"psPsm{i}", tag=f"psPsm{i}")
                       for i in range(2))

        # ---- steady state: attn(h) with head h+2's projection woven in ----
        for h in range(max(0, NH - 2)):
            if h + 3 < NH:
                load_wq(h + 3)
            run_attn(h, proj_gen(h + 2))

        # x is done (last read: head NH-1's projection); its SBUF hosts the
        # c_proj weights, whose DMA overlaps the last two attention heads
        close("xpool")
        wppool = tc.tile_pool(name="wppool", bufs=1, side="right")
        cms["wppool"] = wppool
        wppool_p = wppool.__enter__()
        wps = []
        for hd in range(NH):
            wpt = wppool_p.tile([128, C], BF16, name=f"wp{hd}", tag=f"wp{hd}")
            nc.sync.dma_start(out=wpt[:], in_=wp_d[hd])
            wps.append(wpt)

        # last heads: serial (block-interleaving two attention streams
        # miscompares on the hw path); phase C weaves into head NH-1 only,
        # gated on its query-tile progress (all other heads are done)
        run_attn(NH - 2)
        cg = c_gen(wps)
        c_tb = next(cg)
        for q in attn_gen(NH - 1, defer=False):
            for _ in range(4):
                if c_tb is None or c_tb // JMAX + 1 > q:
                    break
                c_tb = next(cg, None)
        for fn in pending:
            if fn is not None:
                fn()
        pending.clear()
        for _ in cg:
            pass

        close("psPsm", "psO", "psS", "psq",
              "wppool", "wvpool",
              "ropool", "ppool", "qkpool", "vpool",
              "wqpool", "ohpool", "cpool")

    if legalize:
        _legalize_waits(nc)
    return nc


# ---------------------------------------------------------------- host side

_PERM = np.concatenate([np.arange(0, HD, 2), np.arange(1, HD, 2)])  # de-interleave


def _split8(a32):
    """fp8 hi/lo decomposition: a ~= hi + lo with lo the rounding residual."""
    hi = a32.astype(NPF8)
    lo = (a32 - hi.astype(np.float32)).astype(NPF8)
    return hi, lo


def shard_core(core, x, freqs_cos, freqs_sin, Wqkv, bqkv, Wproj,
               T=T, C=C, NH=NH, qtile=256, use_bqkv=False):
    """Build the in_map for one core."""
    CB = C // 128
    DV = NH * 128
    QTILE = min(qtile, T)
    b = core // 2
    hb = (core % 2) * NH

    xt = np.ascontiguousarray(
        x[b].T.reshape(CB, 128, T).transpose(1, 0, 2)).astype(np.float32)
    xth, xtl = _split8(xt)

    # [2, NH, 128] column indices (q/k, de-interleaved within each head)
    cols = (np.arange(2)[:, None, None] * C
            + (hb + np.arange(NH))[None, :, None] * HD + _PERM[None, None, :])
    wqk = Wqkv[:, cols]                              # [C, 2, NH, 128]
    wqk = np.ascontiguousarray(
        wqk.reshape(CB, 128, 2, NH, 128).transpose(2, 3, 1, 0, 4)
        .reshape(2, NH, 128, CB, 128)) * WSCALE
    wqkh, wqkl = _split8(wqk.astype(np.float32))

    wv = np.ascontiguousarray(
        Wqkv[:, 2 * C + hb * HD: 2 * C + (hb + NH) * HD]
        .reshape(CB, 128, DV).transpose(1, 0, 2)) * WSCALE
    wvh, wvl = _split8(wv.astype(np.float32))
    wp = np.ascontiguousarray(
        Wproj[hb * HD:(hb + NH) * HD, :].reshape(NH, 128, C)
        * (1.0 / WSCALE)).astype(NPBF)

    cos2 = np.concatenate([freqs_cos.T, freqs_cos.T], 0)
    cos2 = np.ascontiguousarray(cos2).astype(NPBF)   # [128, T]
    sin2s = np.concatenate([-freqs_sin.T, freqs_sin.T], 0)
    sin2s = np.ascontiguousarray(sin2s).astype(NPBF)

    u = np.arange(2 * QTILE - 128)[None, :]
    p = np.arange(128)[:, None]
    maskbig = (p <= u - (QTILE - 128)).astype(NPBF)

    im = {
        "xt0": xth, "xt1": xtl, "wqk0": wqkh, "wqk1": wqkl,
        "wv0": wvh, "wv1": wvl, "wp": wp,
        "cos2": cos2, "sin2s": sin2s, "maskbig": maskbig,
        "ones128": np.ones((128, 128), NPBF),
    }
    if use_bqkv:
        bqk = np.empty((128, 2 * NH), np.float32)
        for s in range(2):
            for h in range(NH):
                bqk[:, s * NH + h] = bqkv[s * C + (hb + h) * HD + _PERM]
        im["bqk"] = bqk * WSCALE
        im["onecol"] = np.ones((1, 128), NPBF)
        im["bv"] = np.ascontiguousarray(
            bqkv[2 * C + hb * HD: 2 * C + (hb + NH) * HD][None, :]
            * WSCALE).astype(NPBF)
    return im


_CACHE = {}


def _get_program(use_bqkv):
    key = use_bqkv
    if key not in _CACHE:
        _CACHE[key] = build_program(use_bqkv=use_bqkv)
    return _CACHE[key]


def kernel(x, freqs_cos, freqs_sin, Wqkv, bqkv, Wproj, bproj):
    x = np.asarray(x, np.float32)
    freqs_cos = np.asarray(freqs_cos, np.float32)
    freqs_sin = np.asarray(freqs_sin, np.float32)
    Wqkv = np.asarray(Wqkv, np.float32)
    bqkv = np.asarray(bqkv, np.float32)
    Wproj = np.asarray(Wproj, np.float32)
    bproj = np.asarray(bproj, np.float32)

    use_bqkv = bool(np.any(bqkv != 0))
    nc = _get_program(use_bqkv)
    in_maps = [
        shard_core(c, x, freqs_cos, freqs_sin, Wqkv, bqkv, Wproj,
                   use_bqkv=use_bqkv)
        for c in range(NCORES)
    ]
    try:
        res = run_bass_kernel_spmd(nc, in_maps, list(range(NCORES))).results
    except Exception:
        # transient device faults have been observed; retry once
        res = run_bass_kernel_spmd(nc, in_maps, list(range(NCORES))).results

    out = np.empty((B, T, C), np.float32)
    for b in range(B):
        out[b] = (res[2 * b]["out_partial"].astype(np.float32)
                  + res[2 * b + 1]["out_partial"].astype(np.float32))
    out += bproj[None, None, :]
    return out



# revision 34
# speedup vs baseline: 1.1880x; 1.0426x over previous
"""Causal self-attention (B=4, T=2048, C=2048, H=16, HD=128) on 8 trn2 cores.

Sharding: core c handles batch b = c//2 and heads (c%2)*8 .. +8.
  - QKV projection column-sharded by head, attention head-sharded,
    c_proj row-sharded; the pair partial sums are combined on host.

v3: all matmul operands bf16 (same PE rate as f32r in the cost model,
half the DMA/SBUF), fully SBUF-resident pipeline (no q/k/v DRAM spill),
fused per-head schedule: project head h+1 while attention for head h
runs on the PE; RoPE on DVE overlaps attention; flash tiles of 256
queries to trim the causal diagonal.

v4: QKV/V projections in fp8e4 DoubleRow (0.5 cycles/row, K=256 per
instruction): x and the weights are split hi/lo into fp8 pairs ON THE
HOST (x ~= x_hi + x_lo with x_lo the fp8-rounding residual), and each
K-pair block issues three DoubleRow matmuls (hi*hi + lo*hi + hi*lo)
into the same PSUM group -- 0.75x the bf16 instruction cost at bf16-or-
better accuracy.  Weights are pre-scaled x64 so w_hi avoids the fp8
subnormal floor; the scale cancels via the exp() scale (q*k -> /4096)
and a host-side /64 on Wproj (v path).  Attention (S, O) and c_proj
stay bf16: their operands are produced on device, and splitting them
would cost more DVE/ACT time than the PE time saved.

Self-contained: hardcodes shapes; builds one SPMD Bass program and runs
it on cores 0-7 via run_bass_kernel_spmd.
"""
import math

import ml_dtypes
import numpy as np

import concourse.bass as bass
import concourse.bass_isa as bass_isa
import concourse.library_config as library_config
import concourse.mybir as mybir
import concourse.tile as tile
from concourse.bass_utils import run_bass_kernel_spmd

F32 = mybir.dt.float32
BF16 = mybir.dt.bfloat16
FP8 = mybir.dt.float8e4
AF = mybir.ActivationFunctionType
ALU = mybir.AluOpType
DR = mybir.MatmulPerfMode.DoubleRow
NPBF = ml_dtypes.bfloat16
NPF8 = mybir.dt.np(FP8)
WSCALE = 64.0            # fp8 weight pre-scale (keeps w_hi out of subnormals)

# problem dims
B, T, C, H = 4, 2048, 2048, 16
HD = 128
NCORES = 8
NH = H // 2          # heads per core

_ctr = [0]


def _legalize_waits(nc, max_waits=1):
    """This walrus build rejects >1 sync wait per instruction. Hoist extra
    waits onto same-engine NoOps inserted directly before the instruction."""
    n_split = 0
    for f in nc.m.functions:
        for blk in f.blocks:
            newil = []
            changed = False
            for inst in blk.instructions:
                si = inst.sync_info
                if si is not None and si.on_wait and len(si.on_wait) > max_waits:
                    waits = list(si.on_wait)
                    for w in waits[:-max_waits]:
                        _ctr[0] += 1
                        nop = mybir.InstNoOp(name=f"I-waitfix-{_ctr[0]}")
                        nop.engine = inst.engine
                        nop.sync_info = mybir.SyncInfo(on_wait=[w], on_update=[])
                        newil.append(nop)
                    inst.sync_info = mybir.SyncInfo(
                        on_wait=waits[-max_waits:], on_update=list(si.on_update)
                    )
                    changed = True
                    n_split += 1
                newil.append(inst)
            if changed:
                blk.instructions = newil
    return n_split


def build_program(T=T, C=C, NH=NH, use_bqkv=False, qtile=256, legalize=True):
    """One core's program: full pipeline for (1 batch, NH heads)."""
    CB = C // 128          # contraction blocks
    TBn = T // 128         # token blocks
    QTILE = min(qtile, T)  # flash query-tile
    NQT = T // QTILE
    JMAX = QTILE // 128
    DV = NH * 128          # v/proj-shard width
    NG = max(1, DV // 512)  # v-projection head groups (512 cols each)
    GW = DV // NG           # group width
    GH = NH // NG           # heads per group
    TCH = 512               # xt column chunk / qk psq chunk
    NTC = T // TCH
    inv_sqrt_hd = 1.0 / math.sqrt(HD)

    inv_sqrt_hd /= WSCALE * WSCALE   # q and k each carry a x64 weight scale

    nc = bass.Bass()
    xt_d = [nc.dram_tensor(f"xt{p}", [128, CB, T], FP8, kind="ExternalInput")
            for p in range(2)]
    wqk_d = [nc.dram_tensor(f"wqk{p}", [2, NH, 128, CB, 128], FP8,
                            kind="ExternalInput")
             for p in range(2)]
    wv_d = [nc.dram_tensor(f"wv{p}", [128, CB, DV], FP8, kind="ExternalInput")
            for p in range(2)]
    wp_d = nc.dram_tensor("wp", [NH, 128, C], BF16, kind="ExternalInput")
    cos2_d = nc.dram_tensor("cos2", [128, T], BF16, kind="ExternalInput")
    sin2s_d = nc.dram_tensor("sin2s", [128, T], BF16, kind="ExternalInput")
    mask_d = nc.dram_tensor("maskbig", [128, 2 * QTILE - 128], BF16, kind="ExternalInput")
    ones_d = nc.dram_tensor("ones128", [128, 128], BF16, kind="ExternalInput")
    if use_bqkv:
        # [128, 2*NH] per-partition q/k bias columns; V bias via rank-1 matmul
        bqk_d = nc.dram_tensor("bqk", [128, 2 * NH], F32, kind="ExternalInput")
        onecol_d = nc.dram_tensor("onecol", [1, 128], BF16, kind="ExternalInput")
        bv_d = nc.dram_tensor("bv", [1, DV], BF16, kind="ExternalInput")
    out_d = nc.dram_tensor("out_partial", [T, C], BF16, kind="ExternalOutput")

    with tile.TileContext(nc) as tc:
        cms = {}

        def openpool(name, **kw):
            cm = tc.tile_pool(name=name, bufs=1, **kw)
            cms[name] = cm
            return cm.__enter__()

        def close(*names):
            for n in names:
                cms.pop(n).__exit__(None, None, None)

        # ---- pools + tiles up front, in per-side stack order.
        # left SBUF stack (live to the end): cpool..ropool;
        # right stack: xpool, later replaced by wppool.
        cpool = openpool("cpool")
        cos2 = cpool.tile([128, T], BF16, name="cos2")
        sin2s = cpool.tile([128, T], BF16, name="sin2s")
        maskt = cpool.tile([128, 2 * QTILE - 128], BF16, name="maskt")
        ones = cpool.tile([128, 128], BF16, name="ones")
        if use_bqkv:
            bqk = cpool.tile([128, 2 * NH], F32, name="bqk")
            onecol = cpool.tile([1, 128], BF16, name="onecol")
            bv = cpool.tile([1, DV], BF16, name="bv")

        outc_r = [cpool.tile([128, 512], BF16, name=f"outc{i}", tag=f"outc{i}")
                  for i in range(2)]

        ohpool = openpool("ohpool")
        ohs = [ohpool.tile([128, T], BF16, name=f"oh{h}", tag=f"oh{h}")
               for h in range(NH)]

        wqpool = openpool("wqpool")
        # wq_r[slot][side(q/k)][part(hi/lo)] -> [128, CB, 128] fp8 tile
        wq_r = [[[wqpool.tile([128, CB, 128], FP8, name=f"w{s}{i}p{p}",
                              tag=f"w{s}{i}p{p}") for p in range(2)]
                 for s in range(2)] for i in range(2)]

        vpool = openpool("vpool")
        vgs = [[vpool.tile([128, GW], BF16, name=f"vg{g}_{tb}", tag=f"vg{g}_{tb}")
                for tb in range(TBn)] for g in range(NG)]

        qkpool = openpool("qkpool")
        qk_r = [(qkpool.tile([128, T], BF16, name=f"qr{i}", tag=f"qr{i}"),
                 qkpool.tile([128, T], BF16, name=f"kr{i}", tag=f"kr{i}"))
                for i in range(3)]

        ppool = openpool("ppool")
        qb_r = [ppool.tile([128, TCH], BF16, name=f"qb{i}", tag=f"qb{i}")
                for i in range(3)]
        qrot_r = [ppool.tile([128, TCH], BF16, name=f"qrot{i}", tag=f"qrot{i}")
                  for i in range(3)]
        pt_r = [ppool.tile([128, 2 * QTILE], BF16, name=f"pt{i}", tag=f"pt{i}")
                for i in range(3)]

        ropool = openpool("ropool")
        rec_r = [ropool.tile([128, QTILE], F32, name="rec0", tag="rec0")]
        racc_r = [ropool.tile([128, QTILE], BF16, name=f"racc{i}", tag=f"racc{i}")
                  for i in range(2)]


        wvpool = openpool("wvpool", side="right")
        wvg_t = [wvpool.tile([128, CB, GW], FP8, name=f"wvg{p}") for p in range(2)]

        # x lives in per-chunk pools so the first two chunks can be released
        # mid-attention (right after the last head's projection reads them)
        # and their SBUF reused for the c_proj weights, whose DMA then
        # overlaps attention instead of stalling phase C.
        # Right-stack order: wvpool | xp3 | xp2 | xp01(top, closes first).
        xchunks = [None] * NTC
        for tc_ in range(NTC - 1, 1, -1):
            xp = openpool(f"xp{tc_}", side="right")
            xchunks[tc_] = [xp.tile([128, CB, TCH], FP8, name=f"xb{tc_}_{p}")
                            for p in range(2)]
        xp01 = openpool("xp01", side="right")
        for tc_ in range(min(2, NTC)):
            xchunks[tc_] = [xp01.tile([128, CB, TCH], FP8, name=f"xb{tc_}_{p}")
                            for p in range(2)]

        # PSUM: prologue uses psq(2)+psv(2); psv then closes and the
        # attention pools take its banks -> psq2+psS2+psO2+psR2 = 8 banks.
        psqp = openpool("psq", space="PSUM")
        psq_r = [psqp.tile([128, TCH], F32, name=f"psq{i}", tag=f"psq{i}")
                 for i in range(2)]
        psvp = openpool("psv", space="PSUM")
        psv_r = [psvp.tile([128, GW], F32, name=f"psv{i}", tag=f"psv{i}")
                 for i in range(2)]
        psS_r, psO_r = [], []

        ctr = {"psv": 0, "psq": 0, "psS": 0, "psO": 0, "qb": 0, "pt": 0,
               "rec": 0, "racc": 0, "psPsm": 0, "outc": 0}

        def ring(rs, key):
            t = rs[ctr[key] % len(rs)]
            ctr[key] += 1
            return t

        # ---------------- DMA preloads (issue order = queue order) ----------
        def load_wq(h):
            for s in range(2):
                for p in range(2):
                    nc.sync.dma_start(out=wq_r[h % 2][s][p][:],
                                      in_=wqk_d[p][s, h])

        def load_wvg(g):
            h2 = CB // 2
            for half in (slice(0, h2), slice(h2, CB)):
                for p in range(2):
                    nc.sync.dma_start(out=wvg_t[p][:, half, :],
                                      in_=wv_d[p][:, half, g * GW:(g + 1) * GW])

        def load_xt_chunk(tc_, fine=False):
            ts = slice(tc_ * TCH, (tc_ + 1) * TCH)
            if fine:
                # 4-cb pieces, hi/lo interleaved: compute starts as each lands
                for cb in range(0, CB, 4):
                    for p in range(2):
                        nc.sync.dma_start(out=xchunks[tc_][p][:, cb:cb + 4, :],
                                          in_=xt_d[p][:, cb:cb + 4, ts])
            else:
                for p in range(2):
                    nc.sync.dma_start(out=xchunks[tc_][p][:, :, :],
                                      in_=xt_d[p][:, :, ts])

        # issue order tracks the prologue's consumption order (see below):
        # chunk-0 q/k matmuls interleave per cb-pair, then V group 0.
        nc.sync.dma_start(out=wq_r[0][0][0][:], in_=wqk_d[0][0, 0])
        nc.sync.dma_start(out=xchunks[0][0][:, 0:4, :], in_=xt_d[0][:, 0:4, 0:TCH])
        nc.sync.dma_start(out=wq_r[0][0][1][:], in_=wqk_d[1][0, 0])
        nc.sync.dma_start(out=xchunks[0][1][:, 0:4, :], in_=xt_d[1][:, 0:4, 0:TCH])
        for p in range(2):
            nc.sync.dma_start(out=wq_r[0][1][p][:], in_=wqk_d[p][1, 0])
        for cb in range(4, CB, 4):
            for p in range(2):
                nc.sync.dma_start(out=xchunks[0][p][:, cb:cb + 4, :],
                                  in_=xt_d[p][:, cb:cb + 4, 0:TCH])
        load_wvg(0)
        nc.sync.dma_start(out=cos2[:], in_=cos2_d[:])
        nc.sync.dma_start(out=sin2s[:], in_=sin2s_d[:])
        if NTC > 1:
            load_xt_chunk(1)
        nc.sync.dma_start(out=ones[:], in_=ones_d[:])
        nc.sync.dma_start(out=maskt[:], in_=mask_d[:])
        for tc_ in range(2, NTC):
            load_xt_chunk(tc_)
        load_wq(1)
        if use_bqkv:
            nc.sync.dma_start(out=bqk[:], in_=bqk_d[:])
            nc.sync.dma_start(out=onecol[:], in_=onecol_d[:])
            nc.sync.dma_start(out=bv[:], in_=bv_d[:])

        # ---------------- building blocks ----------------
        # fp8 DoubleRow term order per K-pair: hi*hi, lo*hi, hi*lo
        TERMS = ((0, 0), (1, 0), (0, 1))

        TBC = TCH // 128          # token blocks per x chunk

        def vproj_group(g, tb0, tb1):
            """V columns for head group g, token blocks [tb0, tb1)."""
            for tb in range(tb0, tb1):
                xc, col = xchunks[tb // TBC], (tb % TBC) * 128
                psv = ring(psv_r, "psv")
                for cp in range(CB // 2):
                    for ti, (px, pw) in enumerate(TERMS):
                        nc.tensor.matmul(
                            psv[:],
                            xc[px][:, 2 * cp:2 * cp + 2, col:col + 128],
                            wvg_t[pw][:, 2 * cp:2 * cp + 2, :],
                            start=(cp == 0 and ti == 0),
                            stop=(cp == CB // 2 - 1 and ti == 2 and not use_bqkv),
                            perf_mode=DR)
                if use_bqkv:
                    nc.tensor.matmul(psv[:], onecol[:], bv[:, g * GW:(g + 1) * GW],
                                     start=False, stop=True)
                nc.scalar.copy(out=vgs[g][tb][:], in_=psv[:])

        def rope_tail(h, s, tc_, ps, dmae=None):
            """PSUM chunk -> RoPE -> qr/kr slice (ACT+DMA+DVE, no PE work).
            During the prologue the SP queue is congested with preloads ->
            swaps ride the idle gpsimd queue; in steady state the Pool SWDGE
            path is ~2x slower per swap than HWDGE, so use SP."""
            dst = qk_r[h % 3][s]
            ts = slice(tc_ * TCH, (tc_ + 1) * TCH)
            qb = ring(qb_r, "qb")
            qrot = qrot_r[(ctr["qb"] - 1) % len(qrot_r)]
            if use_bqkv:
                nc.vector.tensor_scalar(
                    qb[:], ps[:], bqk[:, s * NH + h:s * NH + h + 1], None, ALU.add)
            else:
                nc.scalar.copy(out=qb[:], in_=ps[:])
            dmae = dmae or nc.sync
            dmae.dma_start(out=qrot[0:64, :], in_=qb[64:128, :])
            dmae.dma_start(out=qrot[64:128, :], in_=qb[0:64, :])
            nc.vector.tensor_mul(qb[:], qb[:], cos2[:, ts])
            nc.vector.tensor_mul(qrot[:], qrot[:], sin2s[:, ts])
            nc.vector.tensor_add(dst[:, ts], qb[:], qrot[:])

        def qk_matmuls(h, s, tc_, ps):
            xc = xchunks[tc_]
            for cp in range(CB // 2):
                for ti, (px, pw) in enumerate(TERMS):
                    nc.tensor.matmul(
                        ps[:], wq_r[h % 2][s][pw][:, 2 * cp:2 * cp + 2, :],
                        xc[px][:, 2 * cp:2 * cp + 2, :],
                        start=(cp == 0 and ti == 0),
                        stop=(cp == CB // 2 - 1 and ti == 2),
                        perf_mode=DR)
                    yield

        def qkproj_chunk(h, s, tc_):
            """psq for (head h, q/k s), token chunk tc_, then RoPE.
            (prologue-only path: swaps ride the idle gpsimd queue)"""
            ps = ring(psq_r, "psq")
            for _ in qk_matmuls(h, s, tc_, ps):
                pass
            rope_tail(h, s, tc_, ps, dmae=nc.gpsimd)

        def proj_gen(h, on_chunk=None):
            """Generator form of a head's projection: yields after each PE
            matmul so the work can be woven into an attention stream.
            on_chunk(tc_) fires after both q/k rope tails of chunk tc_."""
            for tc_ in range(NTC):
                for s in (0, 1):
                    ps = ring(psq_r, "psq")
                    yield from qk_matmuls(h, s, tc_, ps)
                    rope_tail(h, s, tc_, ps)
                if on_chunk is not None:
                    on_chunk(tc_)

        pending = []   # deferred per-qt R matmuls + norms, shared across heads

        def attn_gen(h, defer=True):
            """Generator: one flash-attention head; yields after each key
            block PAIR.  Each pair's two S matmuls share one full PSUM bank
            (two accumulation-group halves) so exp runs on a [128, 2*QTILE]
            tile -- ~23% less ACT time than per-block exps, which keeps the
            in-order ACT queue from delaying the O-feed chain.  S pairs run
            LOOKAHEAD pairs ahead of their O consumers so the exp result is
            ready before the PE needs it -- the PE must run back-to-back to
            hold its top p-state.

            Softmax denominators never touch the PE until a single per-qt
            ones-matmul: exp tiles accumulate via bf16 DVE adds into racc."""
            LOOKAHEAD = 2
            qr, kr = qk_r[h % 3]
            g, hi = divmod(h, GH)
            vg = vgs[g]

            for qt in range(NQT):
                ntk = (qt + 1) * JMAX
                npair = ntk // 2
                tqs = slice(qt * QTILE, (qt + 1) * QTILE)
                psO = ring(psO_r, "psO")
                racc = ring(racc_r, "racc")
                pts = {}

                def emit_S(pr, ntk=ntk, qt=qt, pts=pts, racc=racc):
                    psS = ring(psS_r, "psS")
                    for half in range(2):
                        tkb = 2 * pr + half
                        nc.tensor.matmul(
                            psS[:, half * QTILE:(half + 1) * QTILE],
                            kr[:, tkb * 128:(tkb + 1) * 128], qr[:, tqs],
                            start=(half == 0), stop=(half == 1),
                            skip_group_check=True)
                    pt = ring(pt_r, "pt")
                    nc.scalar.activation(pt[:], psS[:], AF.Exp, scale=inv_sqrt_hd)
                    halves = []
                    for half in range(2):
                        tkb = 2 * pr + half
                        ph = pt[:, half * QTILE:(half + 1) * QTILE]
                        j = tkb - qt * JMAX
                        if j >= 0:
                            m0 = (JMAX - 1 - j) * 128
                            nc.vector.tensor_mul(ph, ph, maskt[:, m0:m0 + QTILE])
                        halves.append(ph)
                    if pr == 0:
                        nc.vector.tensor_add(racc[:], halves[0], halves[1])
                    else:
                        nc.vector.tensor_add(racc[:], racc[:], halves[0])
                        nc.vector.tensor_add(racc[:], racc[:], halves[1])
                    pts[pr] = pt

                def emit_O(pr, ntk=ntk, pts=pts, psO=psO):
                    pt = pts[pr]
                    for half in range(2):
                        tkb = 2 * pr + half
                        nc.tensor.matmul(
                            psO[:], vg[tkb][:, hi * 128:(hi + 1) * 128],
                            pt[:, half * QTILE:(half + 1) * QTILE],
                            start=(tkb == 0), stop=(tkb == ntk - 1))

                def pop_pending():
                    # paired yields are half as frequent as v3's per-block
                    # yields: pop two entries per yield so a deferred
                    # denominator never outlives its racc/psO ring slot
                    for _ in range(2):
                        if pending and (fn := pending.pop(0)) is not None:
                            fn()

                for pr in range(npair):
                    emit_S(pr)
                    if pr >= LOOKAHEAD:
                        emit_O(pr - LOOKAHEAD)
                    pop_pending()
                    yield qt
                for pr in range(max(0, npair - LOOKAHEAD), npair):
                    emit_O(pr)
                    pop_pending()
                    yield qt

                def denom_tail(racc=racc, psO=psO, tqs=tqs):
                    # single ones-matmul on the accumulated exp tile gives the
                    # denominator broadcast across partitions (psPsm banks are
                    # free outside the phase-C stream)
                    rP = ring(psPsm_r, "psPsm")
                    nc.tensor.matmul(rP[:, 0:QTILE], ones[:], racc[:],
                                     start=True, stop=True)
                    rec = ring(rec_r, "rec")
                    nc.vector.reciprocal(rec[:], rP[:, 0:QTILE])
                    nc.vector.tensor_mul(ohs[h][:, tqs], psO[:], rec[:])

                if defer:
                    pending.extend([None] * min(2, max(1, npair // 3))
                                   + [denom_tail])
                else:
                    denom_tail()

        def run_attn(h, filler=None):
            """Emit attention head h, weaving in filler matmuls (3 per key
            block, so the woven projection finishes ~2/3 through the head and
            its rope tail drains before the next head's attention reads it)."""
            for i, _ in enumerate(attn_gen(h, defer=True)):
                if filler is not None:
                    for _ in range(4):
                        if next(filler, "done") == "done":
                            filler = None
                            break
            if filler is not None:
                for _ in filler:
                    pass

        psPsm_r = []

        def c_gen(wps):
            """Phase C emitter: psP chunk groups of 8 matmuls, then copy+DMA
            of that [128,512] output chunk. Yields its tb before each PE op."""
            for tb in range(TBn):
                rs = slice(tb * 128, (tb + 1) * 128)
                for c0 in range(0, C, 512):
                    psp = ring(psPsm_r, "psPsm")
                    for hd in range(NH):
                        yield tb
                        nc.tensor.matmul(
                            psp[:], ohs[hd][:, rs], wps[hd][:, c0:c0 + 512],
                            start=(hd == 0), stop=(hd == NH - 1))
                    ob = ring(outc_r, "outc")
                    nc.scalar.copy(out=ob[:], in_=psp[:])
                    nc.sync.dma_start(out=out_d[rs, c0:c0 + 512], in_=ob[:])

        def run_attn_pair_with_c(h0, h1, cg):
            """Interleave the last two attention heads block-by-block (each
            absorbs the other's softmax latency) and weave in phase-C chunk
            matmuls for query tiles whose normalization is already done."""
            g0, g1 = attn_gen(h0, defer=False), attn_gen(h1, defer=False)
            q0 = q1 = -1
            c_tb = next(cg)          # tb of the NEXT pending C matmul
            while True:
                step = False
                if q0 is not None:
                    q0 = next(g0, None)
                    step = step or q0 is not None
                if q1 is not None:
                    q1 = next(g1, None)
                    step = step or q1 is not None
                if not step:
                    break
                # pair norms are inline: qt i fully normed once both
                # generators are past it
                qmin = min(q0 if q0 is not None else NQT,
                           q1 if q1 is not None else NQT)
                for _ in range(4):
                    if c_tb is None or c_tb // JMAX + 1 > qmin:
                        break
                    c_tb = next(cg, None)
            # flush deferred norms, then drain the rest of phase C
            for fn in pending:
                if fn is not None:
                    fn()
            pending.clear()
            for _ in cg:
                pass

        # ---------------- prologue: head 0 projection + all V groups --------
        tbpg = TBn // NTC
        # chunk 0: q/k interleaved per cb-pair to track piecewise x arrival
        psA = ring(psq_r, "psq")
        psB = ring(psq_r, "psq")
        gA = qk_matmuls(0, 0, 0, psA)
        gB = qk_matmuls(0, 1, 0, psB)
        for _ in range(CB // 2):
            for _ in range(3):
                next(gA)
            for _ in range(3):
                next(gB)
        rope_tail(0, 0, 0, psA, dmae=nc.gpsimd)
        rope_tail(0, 1, 0, psB, dmae=nc.gpsimd)
        vproj_group(0, 0, tbpg)
        for tc_ in range(1, NTC):
            qkproj_chunk(0, 0, tc_)
            qkproj_chunk(0, 1, tc_)
            vproj_group(0, tc_ * tbpg, (tc_ + 1) * tbpg)
        pg_next = proj_gen(1)
        if NG > 1:
            load_wvg(1)
            # cover the wvg reload with the first chunks of head 1's proj
            for _ in range(2 * 2 * (CB // 2) * 3):
                if next(pg_next, None) is None:
                    break
            for tc_ in range(NTC):
                vproj_group(1, tc_ * tbpg, (tc_ + 1) * tbpg)

        # swap psv banks for the attention accumulators + small phase-C psP:
        # psq(2) + psS(2) + psO(2) + psPsm(2) = 8 banks, static to the end
        close("psv")
        psSp = openpool("psS", space="PSUM")
        psS_r.extend(psSp.tile([128, 2 * QTILE], F32, name=f"psS{i}",
                               tag=f"psS{i}") for i in range(2))
        psOp = openpool("psO", space="PSUM")
        psO_r.extend(psOp.tile([128, QTILE], F32, name=f"psO{i}", tag=f"psO{i}")
                     for i in range(2))
        psPp2 = openpool("psPsm", space="PSUM")
        psPsm_r.extend(psPp2.tile([128, 512], F32, name=f"psPsm{i}", tag=f"psPsm{i}")
                       for i in range(2))

        # ---- steady state: attn(h) with head h+1's projection woven in ----
        wps = []

        def release_x_load_wp(tc_):
            # once the LAST head's projection has consumed x chunks 0-1,
            # their SBUF hosts the c_proj weights; the DMA overlaps the
            # remaining attention heads instead of stalling phase C.
            if tc_ != 1:
                return
            close("xp01")
            wpp = openpool("wppool", side="right")
            for hd in range(NH):
                wpt = wpp.tile([128, C], BF16, name=f"wp{hd}", tag=f"wp{hd}")
                # Pool SWDGE queue: keeps the 8-transfer burst off HWDGE,
                # which the woven projection's rope swaps are latency-bound on
                nc.gpsimd.dma_start(out=wpt[:], in_=wp_d[hd])
                wps.append(wpt)

        for h in range(NH - 1):
            if h + 2 < NH:
                load_wq(h + 2)
            run_attn(h, pg_next)
            nh = h + 2
            if nh == NH - 1:
                pg_next = proj_gen(nh, on_chunk=release_x_load_wp)
            elif nh < NH:
                pg_next = proj_gen(nh)
            else:
                pg_next = None

        # last head: phase C weaves into head NH-1, gated on its query-tile
        # progress (all other heads are done)
        cg = c_gen(wps)
        c_tb = next(cg)
        for q in attn_gen(NH - 1, defer=False):
            for _ in range(4):
                if c_tb is None or c_tb // JMAX + 1 > q:
                    break
                c_tb = next(cg, None)
        for fn in pending:
            if fn is not None:
                fn()
        pending.clear()
        for _ in cg:
            pass

        close("psPsm", "psO", "psS", "psq", "wppool",
              *[f"xp{t}" for t in range(2, NTC)],
              "wvpool",
              "ropool", "ppool", "qkpool", "vpool",
              "wqpool", "ohpool", "cpool")

    if legalize:
        _legalize_waits(nc)
    return nc


# ---------------------------------------------------------------- host side

_PERM = np.concatenate([np.arange(0, HD, 2), np.arange(1, HD, 2)])  # de-interleave


def _split8(a32):
    """fp8 hi/lo decomposition: a ~= hi + lo with lo the rounding residual."""
    hi = a32.astype(NPF8)
    lo = (a32 - hi.astype(np.float32)).astype(NPF8)
    return hi, lo


def shard_core(core, x, freqs_cos, freqs_sin, Wqkv, bqkv, Wproj,
               T=T, C=C, NH=NH, qtile=256, use_bqkv=False):
    """Build the in_map for one core."""
    CB = C // 128
    DV = NH * 128
    QTILE = min(qtile, T)
    b = core // 2
    hb = (core % 2) * NH

    xt = np.ascontiguousarray(
        x[b].T.reshape(CB, 128, T).transpose(1, 0, 2)).astype(np.float32)
    xth, xtl = _split8(xt)

    # [2, NH, 128] column indices (q/k, de-interleaved within each head)
    cols = (np.arange(2)[:, None, None] * C
            + (hb + np.arange(NH))[None, :, None] * HD + _PERM[None, None, :])
    wqk = Wqkv[:, cols]                              # [C, 2, NH, 128]
    wqk = np.ascontiguousarray(
        wqk.reshape(CB, 128, 2, NH, 128).transpose(2, 3, 1, 0, 4)
        .reshape(2, NH, 128, CB, 128)) * WSCALE
    wqkh, wqkl = _split8(wqk.astype(np.float32))

    wv = np.ascontiguousarray(
        Wqkv[:, 2 * C + hb * HD: 2 * C + (hb + NH) * HD]
        .reshape(CB, 128, DV).transpose(1, 0, 2)) * WSCALE
    wvh, wvl = _split8(wv.astype(np.float32))
    wp = np.ascontiguousarray(
        Wproj[hb * HD:(hb + NH) * HD, :].reshape(NH, 128, C)
        * (1.0 / WSCALE)).astype(NPBF)

    cos2 = np.concatenate([freqs_cos.T, freqs_cos.T], 0)
    cos2 = np.ascontiguousarray(cos2).astype(NPBF)   # [128, T]
    sin2s = np.concatenate([-freqs_sin.T, freqs_sin.T], 0)
    sin2s = np.ascontiguousarray(sin2s).astype(NPBF)

    u = np.arange(2 * QTILE - 128)[None, :]
    p = np.arange(128)[:, None]
    maskbig = (p <= u - (QTILE - 128)).astype(NPBF)

    im = {
        "xt0": xth, "xt1": xtl, "wqk0": wqkh, "wqk1": wqkl,
        "wv0": wvh, "wv1": wvl, "wp": wp,
        "cos2": cos2, "sin2s": sin2s, "maskbig": maskbig,
        "ones128": np.ones((128, 128), NPBF),
    }
    if use_bqkv:
        bqk = np.empty((128, 2 * NH), np.float32)
        for s in range(2):
            for h in range(NH):
                bqk[:, s * NH + h] = bqkv[s * C + (hb + h) * HD + _PERM]
        im["bqk"] = bqk * WSCALE
        im["onecol"] = np.ones((1, 128), NPBF)
        im["bv"] = np.ascontiguousarray(
            bqkv[2 * C + hb * HD: 2 * C + (hb + NH) * HD][None, :]
            * WSCALE).astype(NPBF)
    return im


_CACHE = {}


def _get_program(use_bqkv):
    key = use_bqkv
    if key not in _CACHE:
        _CACHE[key] = build_program(use_bqkv=use_bqkv)
    return _CACHE[key]


def kernel(x, freqs_cos, freqs_sin, Wqkv, bqkv, Wproj, bproj):
    x = np.asarray(x, np.float32)
    freqs_cos = np.asarray(freqs_cos, np.float32)
    freqs_sin = np.asarray(freqs_sin, np.float32)
    Wqkv = np.asarray(Wqkv, np.float32)
    bqkv = np.asarray(bqkv, np.float32)
    Wproj = np.asarray(Wproj, np.float32)
    bproj = np.asarray(bproj, np.float32)

    use_bqkv = bool(np.any(bqkv != 0))
    nc = _get_program(use_bqkv)
    in_maps = [
        shard_core(c, x, freqs_cos, freqs_sin, Wqkv, bqkv, Wproj,
                   use_bqkv=use_bqkv)
        for c in range(NCORES)
    ]
    try:
        res = run_bass_kernel_spmd(nc, in_maps, list(range(NCORES))).results
    except Exception:
        # transient device faults have been observed; retry once
        res = run_bass_kernel_spmd(nc, in_maps, list(range(NCORES))).results

    out = np.empty((B, T, C), np.float32)
    for b in range(B):
        out[b] = (res[2 * b]["out_partial"].astype(np.float32)
                  + res[2 * b + 1]["out_partial"].astype(np.float32))
    out += bproj[None, None, :]
    return out



# revision 55
# speedup vs baseline: 1.2625x; 1.0627x over previous
"""Causal self-attention (B=4, T=2048, C=2048, H=16, HD=128) on 8 trn2 cores.

Sharding: core c handles batch b = c//2 and heads (c%2)*8 .. +8.
  - QKV projection column-sharded by head, attention head-sharded,
    c_proj row-sharded; the pair partial sums are combined on host.

v3: all matmul operands bf16 (same PE rate as f32r in the cost model,
half the DMA/SBUF), fully SBUF-resident pipeline (no q/k/v DRAM spill),
fused per-head schedule: project head h+1 while attention for head h
runs on the PE; RoPE on DVE overlaps attention; flash tiles of 256
queries to trim the causal diagonal.

v4: QKV/V projections in fp8e4 DoubleRow (0.5 cycles/row, K=256 per
instruction): x and the weights are split hi/lo into fp8 pairs ON THE
HOST (x ~= x_hi + x_lo with x_lo the fp8-rounding residual), and each
K-pair block issues three DoubleRow matmuls (hi*hi + lo*hi + hi*lo)
into the same PSUM group -- 0.75x the bf16 instruction cost at bf16-or-
better accuracy.  Weights are pre-scaled x64 so w_hi avoids the fp8
subnormal floor; the scale cancels via the exp() scale (q*k -> /4096)
and a host-side /64 on Wproj (v path).  Attention (S, O) and c_proj
stay bf16: their operands are produced on device, and splitting them
would cost more DVE/ACT time than the PE time saved.

Self-contained: hardcodes shapes; builds one SPMD Bass program and runs
it on cores 0-7 via run_bass_kernel_spmd.
"""
import math

import ml_dtypes
import numpy as np

import concourse.bass as bass
import concourse.bass_isa as bass_isa
import concourse.library_config as library_config
import concourse.mybir as mybir
import concourse.tile as tile
from concourse.bass_utils import run_bass_kernel_spmd

F32 = mybir.dt.float32
BF16 = mybir.dt.bfloat16
FP8 = mybir.dt.float8e4
AF = mybir.ActivationFunctionType
ALU = mybir.AluOpType
DR = mybir.MatmulPerfMode.DoubleRow
NPBF = ml_dtypes.bfloat16
NPF8 = mybir.dt.np(FP8)
WSCALE = 64.0            # fp8 weight pre-scale (keeps w_hi out of subnormals)

# problem dims
B, T, C, H = 4, 2048, 2048, 16
HD = 128
NCORES = 8
NH = H // 2          # heads per core

_ctr = [0]


def _legalize_waits(nc, max_waits=1):
    """This walrus build rejects >1 sync wait per instruction. Hoist extra
    waits onto same-engine NoOps inserted directly before the instruction."""
    n_split = 0
    for f in nc.m.functions:
        for blk in f.blocks:
            newil = []
            changed = False
            for inst in blk.instructions:
                si = inst.sync_info
                if si is not None and si.on_wait and len(si.on_wait) > max_waits:
                    waits = list(si.on_wait)
                    for w in waits[:-max_waits]:
                        _ctr[0] += 1
                        nop = mybir.InstNoOp(name=f"I-waitfix-{_ctr[0]}")
                        nop.engine = inst.engine
                        nop.sync_info = mybir.SyncInfo(on_wait=[w], on_update=[])
                        newil.append(nop)
                    inst.sync_info = mybir.SyncInfo(
                        on_wait=waits[-max_waits:], on_update=list(si.on_update)
                    )
                    changed = True
                    n_split += 1
                newil.append(inst)
            if changed:
                blk.instructions = newil
    return n_split


def build_program(T=T, C=C, NH=NH, use_bqkv=False, qtile=256, legalize=True,
                  debug=False):
    """One core's program: full pipeline for (1 batch, NH heads)."""
    CB = C // 128          # contraction blocks
    TBn = T // 128         # token blocks
    QTILE = min(qtile, T)  # flash query-tile
    NQT = T // QTILE
    JMAX = QTILE // 128
    DV = NH * 128          # v/proj-shard width
    NG = max(1, DV // 512)  # v-projection head groups (512 cols each)
    GW = DV // NG           # group width
    GH = NH // NG           # heads per group
    TCH = 512               # xt column chunk / qk psq chunk
    NTC = T // TCH
    inv_sqrt_hd = 1.0 / math.sqrt(HD)

    inv_sqrt_hd /= WSCALE * WSCALE   # q and k each carry a x64 weight scale

    nc = bass.Bass()
    xt_d = [nc.dram_tensor(f"xt{p}", [128, CB, T], FP8, kind="ExternalInput")
            for p in range(2)]
    wqk_d = [nc.dram_tensor(f"wqk{p}", [2, NH, 128, CB, 128], FP8,
                            kind="ExternalInput")
             for p in range(2)]
    wv_d = [nc.dram_tensor(f"wv{p}", [128, CB, DV], FP8, kind="ExternalInput")
            for p in range(2)]
    wp_d = [nc.dram_tensor(f"wp{p}", [128, NH, C], FP8, kind="ExternalInput")
            for p in range(2)]
    cos2_d = nc.dram_tensor("cos2", [128, T], BF16, kind="ExternalInput")
    sin2s_d = nc.dram_tensor("sin2s", [128, T], BF16, kind="ExternalInput")
    mask_d = nc.dram_tensor("maskbig", [128, 2 * QTILE - 128], BF16, kind="ExternalInput")
    ones_d = nc.dram_tensor("ones128", [128, 128], BF16, kind="ExternalInput")
    if use_bqkv:
        # [128, 2*NH] per-partition q/k bias columns; V bias via rank-1 matmul
        bqk_d = nc.dram_tensor("bqk", [128, 2 * NH], F32, kind="ExternalInput")
        onecol_d = nc.dram_tensor("onecol", [1, 128], BF16, kind="ExternalInput")
        bv_d = nc.dram_tensor("bv", [1, DV], BF16, kind="ExternalInput")
    out_d = nc.dram_tensor("out_partial", [T, C], BF16, kind="ExternalOutput")

    with tile.TileContext(nc) as tc:
        cms = {}

        def openpool(name, **kw):
            cm = tc.tile_pool(name=name, bufs=1, **kw)
            cms[name] = cm
            return cm.__enter__()

        def close(*names):
            for n in names:
                cms.pop(n).__exit__(None, None, None)

        # ---- pools + tiles up front, in per-side stack order.
        # left SBUF stack (live to the end): cpool..ropool;
        # right stack: xpool, later replaced by wppool.
        cpool = openpool("cpool")
        cos2 = cpool.tile([128, T], BF16, name="cos2")
        sin2s = cpool.tile([128, T], BF16, name="sin2s")
        maskt = cpool.tile([128, 2 * QTILE - 128], BF16, name="maskt")
        ones = cpool.tile([128, 128], BF16, name="ones")
        if use_bqkv:
            bqk = cpool.tile([128, 2 * NH], F32, name="bqk")
            onecol = cpool.tile([1, 128], BF16, name="onecol")
            bv = cpool.tile([1, DV], BF16, name="bv")

        outc_r = [cpool.tile([128, 512], BF16, name=f"outc{i}", tag=f"outc{i}")
                  for i in range(4)]

        # normalized attention outputs, fp8 hi/lo pairs so c_proj can run
        # DoubleRow; one [128, NH, T] tile per part so head PAIRS are
        # addressable as a single [128, 2, *] access pattern
        ohpool = openpool("ohpool")
        ohall = [ohpool.tile([128, NH, T], FP8, name=f"ohall{p}")
                 for p in range(2)]

        wqpool = openpool("wqpool")
        # wq_r[slot][side(q/k)][part(hi/lo)] -> [128, CB, 128] fp8 tile
        wq_r = [[[wqpool.tile([128, CB, 128], FP8, name=f"w{s}{i}p{p}",
                              tag=f"w{s}{i}p{p}") for p in range(2)]
                 for s in range(2)] for i in range(2)]

        vpool = openpool("vpool")
        vgs = [[vpool.tile([128, GW], BF16, name=f"vg{g}_{tb}", tag=f"vg{g}_{tb}")
                for tb in range(TBn)] for g in range(NG)]

        # 2 slots suffice under the h+1 weave: attn(h) reads slot h%2 while
        # the woven projection writes slot (h+1)%2
        qkpool = openpool("qkpool")
        qk_r = [(qkpool.tile([128, T], BF16, name=f"qr{i}", tag=f"qr{i}"),
                 qkpool.tile([128, T], BF16, name=f"kr{i}", tag=f"kr{i}"))
                for i in range(2)]

        ppool = openpool("ppool")
        qb_r = [ppool.tile([128, TCH], BF16, name=f"qb{i}", tag=f"qb{i}")
                for i in range(3)]
        qrot_r = [ppool.tile([128, TCH], BF16, name=f"qrot{i}", tag=f"qrot{i}")
                  for i in range(3)]
        pt_r = [ppool.tile([128, 2 * QTILE], BF16, name=f"pt{i}", tag=f"pt{i}")
                for i in range(4)]

        ropool = openpool("ropool")
        rec_r = [ropool.tile([128, QTILE], F32, name="rec0", tag="rec0")]
        racc_r = [ropool.tile([128, QTILE], BF16, name=f"racc{i}", tag=f"racc{i}")
                  for i in range(2)]
        yb_r = [ropool.tile([128, QTILE], BF16, name=f"yb{i}", tag=f"yb{i}")
                for i in range(2)]


        wvpool = openpool("wvpool", side="right")
        wvg_t = [wvpool.tile([128, CB, GW], FP8, name=f"wvg{p}") for p in range(2)]

        # x lives in per-chunk pools so the first two chunks can be released
        # mid-attention (right after the last head's projection reads them)
        # and their SBUF reused for the c_proj weights, whose DMA then
        # overlaps attention instead of stalling phase C.
        # Right-stack order: wvpool | xp3 | xp2 | xp01(top, closes first).
        xchunks = [None] * NTC
        for tc_ in range(NTC - 1, 1, -1):
            xp = openpool(f"xp{tc_}", side="right")
            xchunks[tc_] = [xp.tile([128, CB, TCH], FP8, name=f"xb{tc_}_{p}")
                            for p in range(2)]
        xp01 = openpool("xp01", side="right")
        for tc_ in range(min(2, NTC)):
            xchunks[tc_] = [xp01.tile([128, CB, TCH], FP8, name=f"xb{tc_}_{p}")
                            for p in range(2)]

        # PSUM: prologue uses psq(2)+psv(2); psv then closes and the
        # attention pools take its banks -> psq2+psS2+psO2+psR2 = 8 banks.
        psqp = openpool("psq", space="PSUM")
        psq_r = [psqp.tile([128, TCH], F32, name=f"psq{i}", tag=f"psq{i}")
                 for i in range(2)]
        psvp = openpool("psv", space="PSUM")
        psv_r = [psvp.tile([128, GW], F32, name=f"psv{i}", tag=f"psv{i}")
                 for i in range(2)]
        psS_r, psO_r = [], []

        ctr = {"psv": 0, "psq": 0, "psS": 0, "psO": 0, "qb": 0, "pt": 0,
               "rec": 0, "racc": 0, "yb": 0, "psPsm": 0, "outc": 0}

        def ring(rs, key):
            t = rs[ctr[key] % len(rs)]
            ctr[key] += 1
            return t

        # ---------------- DMA preloads (issue order = queue order) ----------
        def load_wq(h):
            for s in range(2):
                for p in range(2):
                    nc.sync.dma_start(out=wq_r[h % 2][s][p][:],
                                      in_=wqk_d[p][s, h])

        def load_wvg(g):
            h2 = CB // 2
            for half in (slice(0, h2), slice(h2, CB)):
                for p in range(2):
                    nc.sync.dma_start(out=wvg_t[p][:, half, :],
                                      in_=wv_d[p][:, half, g * GW:(g + 1) * GW])

        def load_xt_chunk(tc_, fine=False):
            ts = slice(tc_ * TCH, (tc_ + 1) * TCH)
            if fine:
                # 4-cb pieces, hi/lo interleaved: compute starts as each lands
                for cb in range(0, CB, 4):
                    for p in range(2):
                        nc.sync.dma_start(out=xchunks[tc_][p][:, cb:cb + 4, :],
                                          in_=xt_d[p][:, cb:cb + 4, ts])
            else:
                for p in range(2):
                    nc.sync.dma_start(out=xchunks[tc_][p][:, :, :],
                                      in_=xt_d[p][:, :, ts])

        # issue order tracks the prologue's consumption order (see below):
        # chunk-0 q/k matmuls interleave per cb-pair, then V group 0.
        nc.sync.dma_start(out=wq_r[0][0][0][:], in_=wqk_d[0][0, 0])
        nc.sync.dma_start(out=xchunks[0][0][:, 0:4, :], in_=xt_d[0][:, 0:4, 0:TCH])
        nc.sync.dma_start(out=xchunks[0][1][:, 0:4, :], in_=xt_d[1][:, 0:4, 0:TCH])
        nc.sync.dma_start(out=wq_r[0][0][1][:], in_=wqk_d[1][0, 0])
        for p in range(2):
            nc.sync.dma_start(out=wq_r[0][1][p][:], in_=wqk_d[p][1, 0])
        for p in range(2):
            nc.sync.dma_start(out=xchunks[0][p][:, 4:CB, :],
                              in_=xt_d[p][:, 4:CB, 0:TCH])
        load_wq(1)
        load_wvg(0)
        if NTC > 1:
            load_xt_chunk(1)
        nc.sync.dma_start(out=cos2[:], in_=cos2_d[:])
        nc.sync.dma_start(out=sin2s[:], in_=sin2s_d[:])
        nc.sync.dma_start(out=ones[:], in_=ones_d[:])
        nc.sync.dma_start(out=maskt[:], in_=mask_d[:])
        for tc_ in range(2, NTC):
            load_xt_chunk(tc_)
        if use_bqkv:
            nc.sync.dma_start(out=bqk[:], in_=bqk_d[:])
            nc.sync.dma_start(out=onecol[:], in_=onecol_d[:])
            nc.sync.dma_start(out=bv[:], in_=bv_d[:])

        # ---------------- building blocks ----------------
        # fp8 DoubleRow term order per K-pair: hi*hi, lo*hi, hi*lo
        TERMS = ((0, 0), (1, 0), (0, 1))

        TBC = TCH // 128          # token blocks per x chunk

        def vproj_group(g, tb0, tb1):
            """V columns for head group g, token blocks [tb0, tb1)."""
            for tb in range(tb0, tb1):
                xc, col = xchunks[tb // TBC], (tb % TBC) * 128
                psv = ring(psv_r, "psv")
                for cp in range(CB // 2):
                    for ti, (px, pw) in enumerate(TERMS):
                        nc.tensor.matmul(
                            psv[:],
                            xc[px][:, 2 * cp:2 * cp + 2, col:col + 128],
                            wvg_t[pw][:, 2 * cp:2 * cp + 2, :],
                            start=(cp == 0 and ti == 0),
                            stop=(cp == CB // 2 - 1 and ti == 2 and not use_bqkv),
                            perf_mode=DR)
                if use_bqkv:
                    nc.tensor.matmul(psv[:], onecol[:], bv[:, g * GW:(g + 1) * GW],
                                     start=False, stop=True)
                nc.scalar.copy(out=vgs[g][tb][:], in_=psv[:])

        def rope_tail(h, s, tc_, ps, dmae=None):
            """PSUM chunk -> RoPE -> qr/kr slice (ACT+DMA+DVE, no PE work).
            During the prologue the SP queue is congested with preloads ->
            swaps ride the idle gpsimd queue; in steady state the Pool SWDGE
            path is ~2x slower per swap than HWDGE, so use SP."""
            dst = qk_r[h % len(qk_r)][s]
            ts = slice(tc_ * TCH, (tc_ + 1) * TCH)
            qb = ring(qb_r, "qb")
            qrot = qrot_r[(ctr["qb"] - 1) % len(qrot_r)]
            if use_bqkv:
                nc.vector.tensor_scalar(
                    qb[:], ps[:], bqk[:, s * NH + h:s * NH + h + 1], None, ALU.add)
            else:
                nc.scalar.copy(out=qb[:], in_=ps[:])
            dmae = dmae or nc.sync
            dmae.dma_start(out=qrot[0:64, :], in_=qb[64:128, :])
            dmae.dma_start(out=qrot[64:128, :], in_=qb[0:64, :])
            nc.vector.tensor_mul(qb[:], qb[:], cos2[:, ts])
            nc.vector.tensor_mul(qrot[:], qrot[:], sin2s[:, ts])
            # the combine is a full head ahead of its consumer: ride the idle
            # Pool engine, freeing DVE for the latency-bound attention chain
            nc.gpsimd.tensor_add(dst[:, ts], qb[:], qrot[:])

        def qk_matmuls(h, s, tc_, ps):
            xc = xchunks[tc_]
            for cp in range(CB // 2):
                for ti, (px, pw) in enumerate(TERMS):
                    nc.tensor.matmul(
                        ps[:], wq_r[h % 2][s][pw][:, 2 * cp:2 * cp + 2, :],
                        xc[px][:, 2 * cp:2 * cp + 2, :],
                        start=(cp == 0 and ti == 0),
                        stop=(cp == CB // 2 - 1 and ti == 2),
                        perf_mode=DR)
                    yield

        def qkproj_chunk(h, s, tc_):
            """psq for (head h, q/k s), token chunk tc_, then RoPE.
            (prologue-only path: swaps ride the idle gpsimd queue)"""
            ps = ring(psq_r, "psq")
            for _ in qk_matmuls(h, s, tc_, ps):
                pass
            rope_tail(h, s, tc_, ps, dmae=nc.gpsimd)

        def proj_gen(h, on_chunk=None):
            """Generator form of a head's projection: yields after each PE
            matmul so the work can be woven into an attention stream.
            on_chunk(tc_) fires after both q/k rope tails of chunk tc_."""
            for tc_ in range(NTC):
                for s in (0, 1):
                    ps = ring(psq_r, "psq")
                    yield from qk_matmuls(h, s, tc_, ps)
                    rope_tail(h, s, tc_, ps)
                if on_chunk is not None:
                    on_chunk(tc_)

        pending = []   # deferred per-qt R matmuls + norms, shared across heads

        def attn_gen(h, defer=True):
            """Generator: one flash-attention head; yields after each key
            block PAIR.  Each pair's two S matmuls share one full PSUM bank
            (two accumulation-group halves) so exp runs on a [128, 2*QTILE]
            tile -- ~23% less ACT time than per-block exps, which keeps the
            in-order ACT queue from delaying the O-feed chain.  S pairs run
            LOOKAHEAD pairs ahead of their O consumers so the exp result is
            ready before the PE needs it -- the PE must run back-to-back to
            hold its top p-state.

            Softmax denominators never touch the PE until a single per-qt
            ones-matmul: exp tiles accumulate via bf16 DVE adds into racc."""
            LOOKAHEAD = 3
            qr, kr = qk_r[h % len(qk_r)]
            g, hi = divmod(h, GH)
            vg = vgs[g]

            for qt in range(NQT):
                ntk = (qt + 1) * JMAX
                npair = ntk // 2
                tqs = slice(qt * QTILE, (qt + 1) * QTILE)
                psO = ring(psO_r, "psO")
                racc = ring(racc_r, "racc")
                pts = {}

                def emit_S(pr, ntk=ntk, qt=qt, pts=pts, racc=racc):
                    psS = ring(psS_r, "psS")
                    for half in range(2):
                        tkb = 2 * pr + half
                        nc.tensor.matmul(
                            psS[:, half * QTILE:(half + 1) * QTILE],
                            kr[:, tkb * 128:(tkb + 1) * 128], qr[:, tqs],
                            start=(half == 0), stop=(half == 1),
                            skip_group_check=True)
                    pt = ring(pt_r, "pt")
                    nc.scalar.activation(pt[:], psS[:], AF.Exp, scale=inv_sqrt_hd)
                    halves = []
                    for half in range(2):
                        tkb = 2 * pr + half
                        ph = pt[:, half * QTILE:(half + 1) * QTILE]
                        j = tkb - qt * JMAX
                        if j >= 0:
                            m0 = (JMAX - 1 - j) * 128
                            nc.vector.tensor_mul(ph, ph, maskt[:, m0:m0 + QTILE])
                        halves.append(ph)
                    if pr == 0:
                        nc.vector.tensor_add(racc[:], halves[0], halves[1])
                    else:
                        nc.vector.tensor_add(racc[:], racc[:], halves[0])
                        nc.vector.tensor_add(racc[:], racc[:], halves[1])
                    pts[pr] = pt

                def emit_O(pr, ntk=ntk, pts=pts, psO=psO):
                    pt = pts[pr]
                    for half in range(2):
                        tkb = 2 * pr + half
                        nc.tensor.matmul(
                            psO[:], vg[tkb][:, hi * 128:(hi + 1) * 128],
                            pt[:, half * QTILE:(half + 1) * QTILE],
                            start=(tkb == 0), stop=(tkb == ntk - 1))

                def pop_pending():
                    # paired yields are half as frequent as v3's per-block
                    # yields: pop two entries per yield so a deferred
                    # denominator never outlives its racc/psO ring slot
                    for _ in range(2):
                        if pending and (fn := pending.pop(0)) is not None:
                            fn()

                for pr in range(npair):
                    emit_S(pr)
                    if pr >= LOOKAHEAD:
                        emit_O(pr - LOOKAHEAD)
                    pop_pending()
                    yield qt
                for pr in range(max(0, npair - LOOKAHEAD), npair):
                    emit_O(pr)
                    pop_pending()
                    yield qt

                def denom_tail(racc=racc, psO=psO, tqs=tqs, h=h):
                    # ones(=4.0)-matmul on the accumulated exp tile gives 4x
                    # the denominator broadcast across partitions (psPsm banks
                    # are free outside the phase-C stream; during the LAST
                    # head phase C rings over psPsm+psq[0], so its
                    # denominators take the idle psq[1] bank); y lands at 16x
                    # scale, inside fp8 range, and is hi/lo split on Pool
                    if h == NH - 1:
                        rP = psq_r[1]
                    else:
                        rP = ring(psPsm_r, "psPsm")
                    nc.tensor.matmul(rP[:, 0:QTILE], ones[:], racc[:],
                                     start=True, stop=True)
                    rec = ring(rec_r, "rec")
                    nc.vector.reciprocal(rec[:], rP[:, 0:QTILE])
                    yb = ring(yb_r, "yb")
                    nc.vector.tensor_mul(yb[:], psO[:], rec[:])
                    nc.gpsimd.tensor_scalar_mul(ohall[0][:, h, tqs], yb[:], 1.0)
                    nc.gpsimd.tensor_sub(ohall[1][:, h, tqs], yb[:],
                                         ohall[0][:, h, tqs])

                if defer:
                    pending.extend([None] * min(2, max(1, npair // 3))
                                   + [denom_tail])
                else:
                    denom_tail()

        def run_attn(h, filler=None):
            """Emit attention head h, weaving in filler matmuls (3 per key
            block, so the woven projection finishes ~2/3 through the head and
            its rope tail drains before the next head's attention reads it)."""
            for i, _ in enumerate(attn_gen(h, defer=True)):
                if filler is not None:
                    for _ in range(4):
                        if next(filler, "done") == "done":
                            filler = None
                            break
            if filler is not None:
                for _ in filler:
                    pass

        psPsm_r = []

        def c_gen(wpt):
            """Phase C emitter: fp8 DoubleRow over head PAIRS (3 hi/lo terms,
            contraction = 2 heads' 256 hd-dims per instruction), then a scaled
            copy (y x16, wp x64 -> /1024) + DMA of that [128,512] output
            chunk. Yields its tb before each PE op."""
            c_banks = psPsm_r + psq_r[:1]   # psq[0] is idle during attn(NH-1)
            for tb in range(TBn):
                rs = slice(tb * 128, (tb + 1) * 128)
                for c0 in range(0, C, 512):
                    psp = c_banks[ctr.setdefault("cps", 0) % len(c_banks)]
                    ctr["cps"] += 1
                    for g4 in range(NH // 2):
                        for ti, (po, pw) in enumerate(TERMS):
                            yield tb
                            nc.tensor.matmul(
                                psp[:], ohall[po][:, 2 * g4:2 * g4 + 2, rs],
                                wpt[pw][:, 2 * g4:2 * g4 + 2, c0:c0 + 512],
                                start=(g4 == 0 and ti == 0),
                                stop=(g4 == NH // 2 - 1 and ti == 2),
                                perf_mode=DR)
                    ob = ring(outc_r, "outc")
                    nc.scalar.mul(ob[:], psp[:], 1.0 / 1024.0)
                    nc.sync.dma_start(out=out_d[rs, c0:c0 + 512], in_=ob[:])

        # ---------------- prologue: head 0 projection + all V groups --------
        tbpg = TBn // NTC
        # chunk 0: q/k interleaved per cb-pair to track piecewise x arrival
        psA = ring(psq_r, "psq")
        psB = ring(psq_r, "psq")
        gA = qk_matmuls(0, 0, 0, psA)
        gB = qk_matmuls(0, 1, 0, psB)
        for _ in range(CB // 2):
            for _ in range(3):
                next(gA)
            for _ in range(3):
                next(gB)
        rope_tail(0, 0, 0, psA, dmae=nc.gpsimd)
        rope_tail(0, 1, 0, psB, dmae=nc.gpsimd)
        # head 1's chunk 0 next: it reuses the already-landed x chunk 0, so
        # the PE stays fed while the V weights and x chunk 1 stream in
        pg_next = proj_gen(1)
        for _ in range(2 * (CB // 2) * 3):
            next(pg_next)
        vproj_group(0, 0, tbpg)
        for tc_ in range(1, NTC):
            qkproj_chunk(0, 0, tc_)
            qkproj_chunk(0, 1, tc_)
            vproj_group(0, tc_ * tbpg, (tc_ + 1) * tbpg)
        if NG > 1:
            load_wvg(1)
            # cover the wvg reload with head 1's chunk-1 projection
            for _ in range(2 * (CB // 2) * 3):
                if next(pg_next, None) is None:
                    break
            for tc_ in range(NTC):
                vproj_group(1, tc_ * tbpg, (tc_ + 1) * tbpg)

        # swap psv banks for the attention accumulators + small phase-C psP:
        # psq(2) + psS(2) + psO(2) + psPsm(2) = 8 banks, static to the end
        close("psv")
        psSp = openpool("psS", space="PSUM")
        psS_r.extend(psSp.tile([128, 2 * QTILE], F32, name=f"psS{i}",
                               tag=f"psS{i}") for i in range(2))
        psOp = openpool("psO", space="PSUM")
        psO_r.extend(psOp.tile([128, QTILE], F32, name=f"psO{i}", tag=f"psO{i}")
                     for i in range(2))
        psPp2 = openpool("psPsm", space="PSUM")
        psPsm_r.extend(psPp2.tile([128, 512], F32, name=f"psPsm{i}", tag=f"psPsm{i}")
                       for i in range(2))

        # ---- steady state: attn(h) with head h+1's projection woven in ----
        wpt = []
        nh2 = NH // 2

        def release_x_load_wp(tc_):
            # once the LAST head's projection has consumed x chunks 0-1,
            # their SBUF hosts the c_proj weights; the DMA overlaps the
            # remaining attention heads instead of stalling phase C.  Issues
            # spread over two chunk boundaries so the burst never backs up
            # the rope-swap DMAs the woven projection is latency-bound on.
            t1, t2 = min(1, NTC - 1), min(2, NTC - 1)
            if tc_ == t1:
                close("xp01")
                wpp = openpool("wppool", side="right")
                wpt.extend(wpp.tile([128, NH, C], FP8, name=f"wpt{p}")
                           for p in range(2))
                for p in range(2):
                    nc.sync.dma_start(out=wpt[p][:, 0:nh2, :],
                                      in_=wp_d[p][:, 0:nh2, :])
            if tc_ == t2:
                for p in range(2):
                    nc.sync.dma_start(out=wpt[p][:, nh2:NH, :],
                                      in_=wp_d[p][:, nh2:NH, :])

        for h in range(NH - 1):
            if h + 2 < NH:
                load_wq(h + 2)
            run_attn(h, pg_next)
            nh = h + 2
            if nh == NH - 1:
                pg_next = proj_gen(nh, on_chunk=release_x_load_wp)
            elif nh < NH:
                pg_next = proj_gen(nh)
            else:
                pg_next = None

        # last head: phase C weaves into head NH-1, gated on its query-tile
        # progress (all other heads are done)
        cg = c_gen(wpt)
        c_tb = next(cg)
        for q in attn_gen(NH - 1, defer=False):
            for _ in range(4):
                if c_tb is None or c_tb // JMAX + 1 > q:
                    break
                c_tb = next(cg, None)
        for fn in pending:
            if fn is not None:
                fn()
        pending.clear()
        for _ in cg:
            pass

        if debug:
            qr_dbg = nc.dram_tensor("qr_dbg", [2, 2, 128, T], BF16, kind="ExternalOutput")
            oh_dbg = nc.dram_tensor("oh_dbg", [2, 128, NH, T], FP8, kind="ExternalOutput")
            vg_dbg = nc.dram_tensor("vg_dbg", [TBn, 128, GW], BF16, kind="ExternalOutput")
            for sl in range(2):
                for s in range(2):
                    nc.sync.dma_start(out=qr_dbg[sl, s], in_=qk_r[sl][s][:])
            for p in range(2):
                nc.sync.dma_start(out=oh_dbg[p], in_=ohall[p][:])
            for tb in range(TBn):
                nc.sync.dma_start(out=vg_dbg[tb], in_=vgs[0][tb][:])
        close("psPsm", "psO", "psS", "psq", "wppool",
              *[f"xp{t}" for t in range(2, NTC)],
              "wvpool",
              "ropool", "ppool", "qkpool", "vpool",
              "wqpool", "ohpool", "cpool")

    if legalize:
        _legalize_waits(nc)
    return nc


# ---------------------------------------------------------------- host side

_PERM = np.concatenate([np.arange(0, HD, 2), np.arange(1, HD, 2)])  # de-interleave


def _split8(a32):
    """fp8 hi/lo decomposition: a ~= hi + lo with lo the rounding residual."""
    hi = a32.astype(NPF8)
    lo = (a32 - hi.astype(np.float32)).astype(NPF8)
    return hi, lo


def shard_core(core, x, freqs_cos, freqs_sin, Wqkv, bqkv, Wproj,
               T=T, C=C, NH=NH, qtile=256, use_bqkv=False):
    """Build the in_map for one core."""
    CB = C // 128
    DV = NH * 128
    QTILE = min(qtile, T)
    b = core // 2
    hb = (core % 2) * NH

    xt = np.ascontiguousarray(
        x[b].T.reshape(CB, 128, T).transpose(1, 0, 2)).astype(np.float32)
    xth, xtl = _split8(xt)

    # [2, NH, 128] column indices (q/k, de-interleaved within each head)
    cols = (np.arange(2)[:, None, None] * C
            + (hb + np.arange(NH))[None, :, None] * HD + _PERM[None, None, :])
    wqk = Wqkv[:, cols]                              # [C, 2, NH, 128]
    wqk = np.ascontiguousarray(
        wqk.reshape(CB, 128, 2, NH, 128).transpose(2, 3, 1, 0, 4)
        .reshape(2, NH, 128, CB, 128)) * WSCALE
    wqkh, wqkl = _split8(wqk.astype(np.float32))

    wv = np.ascontiguousarray(
        Wqkv[:, 2 * C + hb * HD: 2 * C + (hb + NH) * HD]
        .reshape(CB, 128, DV).transpose(1, 0, 2)) * WSCALE
    wvh, wvl = _split8(wv.astype(np.float32))
    wp = np.ascontiguousarray(
        Wproj[hb * HD:(hb + NH) * HD, :].reshape(NH, 128, C)
        .transpose(1, 0, 2)) * WSCALE
    wph, wpl = _split8(wp.astype(np.float32))

    cos2 = np.concatenate([freqs_cos.T, freqs_cos.T], 0)
    cos2 = np.ascontiguousarray(cos2).astype(NPBF)   # [128, T]
    sin2s = np.concatenate([-freqs_sin.T, freqs_sin.T], 0)
    sin2s = np.ascontiguousarray(sin2s).astype(NPBF)

    u = np.arange(2 * QTILE - 128)[None, :]
    p = np.arange(128)[:, None]
    maskbig = (p <= u - (QTILE - 128)).astype(NPBF)

    im = {
        "xt0": xth, "xt1": xtl, "wqk0": wqkh, "wqk1": wqkl,
        "wv0": wvh, "wv1": wvl, "wp0": wph, "wp1": wpl,
        "cos2": cos2, "sin2s": sin2s, "maskbig": maskbig,
        # 4.0: folds a /4 into the softmax denominator so the normalized
        # y lands at 16x scale -- inside fp8 range for the hi/lo split
        "ones128": np.full((128, 128), 4.0, NPBF),
    }
    if use_bqkv:
        bqk = np.empty((128, 2 * NH), np.float32)
        for s in range(2):
            for h in range(NH):
                bqk[:, s * NH + h] = bqkv[s * C + (hb + h) * HD + _PERM]
        im["bqk"] = bqk * WSCALE
        im["onecol"] = np.ones((1, 128), NPBF)
        im["bv"] = np.ascontiguousarray(
            bqkv[2 * C + hb * HD: 2 * C + (hb + NH) * HD][None, :]
            * WSCALE).astype(NPBF)
    return im


_CACHE = {}


def _get_program(use_bqkv):
    key = use_bqkv
    if key not in _CACHE:
        _CACHE[key] = build_program(use_bqkv=use_bqkv)
    return _CACHE[key]


def kernel(x, freqs_cos, freqs_sin, Wqkv, bqkv, Wproj, bproj):
    x = np.asarray(x, np.float32)
    freqs_cos = np.asarray(freqs_cos, np.float32)
    freqs_sin = np.asarray(freqs_sin, np.float32)
    Wqkv = np.asarray(Wqkv, np.float32)
    bqkv = np.asarray(bqkv, np.float32)
    Wproj = np.asarray(Wproj, np.float32)
    bproj = np.asarray(bproj, np.float32)

    use_bqkv = bool(np.any(bqkv != 0))
    nc = _get_program(use_bqkv)
    in_maps = [
        shard_core(c, x, freqs_cos, freqs_sin, Wqkv, bqkv, Wproj,
                   use_bqkv=use_bqkv)
        for c in range(NCORES)
    ]
    try:
        res = run_bass_kernel_spmd(nc, in_maps, list(range(NCORES))).results
    except Exception:
        # transient device faults have been observed; retry once
        res = run_bass_kernel_spmd(nc, in_maps, list(range(NCORES))).results

    out = np.empty((B, T, C), np.float32)
    for b in range(B):
        out[b] = (res[2 * b]["out_partial"].astype(np.float32)
                  + res[2 * b + 1]["out_partial"].astype(np.float32))
    out += bproj[None, None, :]
    return out

